# revision 7
# baseline (speedup 1.0000x reference)
"""MLA (multi-head latent) causal attention on 8 Trainium2 NeuronCores.

Sharding: batch(4) x head-group(2) mesh over 8 cores. Core c handles batch
c//2 and heads [8*(c%2), 8*(c%2)+8). The latent KV projections are small and
recomputed per head-group (an MLA property: the latent KV is shared across
heads). Each core produces a partial output (its head-group's contribution to
y @ wo^T for its batch); the host sums the two partials per batch.

All data is bf16 (rel tolerance 2e-2 leaves plenty of room): this enables the
PE's Fast Weight Load path, halves HBM traffic, and doubles DVE throughput.
PSUM accumulation stays fp32.

Single fused loop over the 4 query chunks of 512 tokens; everything streams
through SBUF (no DRAM scratch):
  per chunk n:
    h-loop (software pipelined): q-proj(h) chain -> RoPE(h) on ACT+DVE,
      scores(h-1) as PAIRS of key blocks into 2-bank PSUM tiles -> one exp
      per pair on ACT -> causal mask as a 0/1 bf16 multiply on the exp
      output (DVE) -> bf16 partial sums of exp on DVE, PV interleaved
      mid-scores, softmax denominator via GPSIMD partition_all_reduce
      (keeps the PE free), 1/z normalize on DVE.
    tail: latent kv proj for chunk n+1 split around the AllGather stage,
      then the output projection with DVE+ACT evacuation in parallel and
      the output DMA split across two queues.

PSUM budget (8 banks): tag "pp" = 3 x [P,2,512] pairs (6 banks, score pairs /
latent halves / out-proj pairs), tag "ps" = 2 x [P,512] singles (q-proj, PV
accumulator, k-up, v-up).
"""

import math
from contextlib import ExitStack

import numpy as np

import concourse.bass as bass
import concourse.mybir as mybir
import concourse.tile as tile
from concourse import bacc
from concourse import bass_isa
from concourse.bass_utils import run_bass_kernel_spmd

# Problem shape (hardcoded per contract).
B, T, C = 4, 2048, 2048
H, D, L = 16, 128, 512
HG = 8           # heads per core
N_CORES = 8
P = 128
KC = C // P      # 16 contraction chunks over C
LC = L // P      # 4 chunks over L
NQ = T // 512    # 4 query chunks of 512
NT = T // P      # 16 key chunks of 128
SCALE = 1.0 / math.sqrt(D)

F32 = mybir.dt.float32
BF16 = mybir.dt.bfloat16
NPBF16 = mybir.dt.np(BF16)

USE_GPSIMD_Z = True   # softmax denominator on GPSIMD instead of a PE matmul

_cached = {}


def _build_program():
    nc = bacc.Bacc()

    xT = nc.dram_tensor("xT", [C, T], BF16, kind="ExternalInput").ap()
    wqT = nc.dram_tensor("wqT", [C, HG * D], BF16, kind="ExternalInput").ap()
    wkvT = nc.dram_tensor("wkvT", [C, L], BF16, kind="ExternalInput").ap()
    wkuT = nc.dram_tensor("wkuT", [L, D], BF16, kind="ExternalInput").ap()
    wvuT = nc.dram_tensor("wvuT", [L, D], BF16, kind="ExternalInput").ap()
    woT = nc.dram_tensor("woT", [HG * D, C], BF16, kind="ExternalInput").ap()
    c2 = nc.dram_tensor("c2", [P, T], BF16, kind="ExternalInput").ap()
    s2 = nc.dram_tensor("s2", [P, T], BF16, kind="ExternalInput").ap()
    outp = nc.dram_tensor("outp", [T, C], BF16, kind="ExternalOutput").ap()
    wku0T = nc.dram_tensor("wku0T", [L, D], BF16, kind="ExternalInput").ap()
    wvu0T = nc.dram_tensor("wvu0T", [L, D], BF16, kind="ExternalInput").ap()
    ccw_s = nc.dram_tensor("ccw_s", [P, 16], BF16, kind="Internal").ap()
    ccw_d = nc.dram_tensor("ccw_d", [2, P, 16], BF16, kind="Internal").ap()
    ccs = [nc.dram_tensor(f"ccs{i}", [L // 2, 512], BF16, kind="Internal").ap()
           for i in range(1, NQ)]
    ccd = [nc.dram_tensor(f"ccd{i}", [2, L // 2, 512], BF16,
                          kind="Internal").ap() for i in range(1, NQ)]

    xT_r = xT.rearrange("(kc p) t -> p kc t", p=P)
    wqT_r = wqT.rearrange("(kc p) e -> p kc e", p=P)
    wkvT_r = wkvT.rearrange("(kc p) l -> p kc l", p=P)
    wkuT_r = wkuT.rearrange("(lc p) d -> p lc d", p=P)
    wvuT_r = wvuT.rearrange("(lc p) d -> p lc d", p=P)
    woT_r = woT.rearrange("(h p) c -> p h c", p=P)

    with tile.TileContext(nc) as tc, ExitStack() as top:
        persist = top.enter_context(tc.tile_pool(name="persist", bufs=1))
        pall = top.enter_context(tc.tile_pool(name="pall", bufs=1, space="PSUM"))
        xpool = top.enter_context(tc.tile_pool(name="xpool", bufs=2))
        kvpool = top.enter_context(tc.tile_pool(name="kvpool", bufs=1))
        rpool = top.enter_context(tc.tile_pool(name="rpool", bufs=2))
        qtp = top.enter_context(tc.tile_pool(name="qtp", bufs=3))
        epool = top.enter_context(tc.tile_pool(name="epool", bufs=2))
        zpool = top.enter_context(tc.tile_pool(name="zpool", bufs=2))
        ypool = top.enter_context(tc.tile_pool(name="ypool", bufs=2))
        opool = top.enter_context(tc.tile_pool(name="opool", bufs=1))

        def pp_tile(name):
            return pall.tile([P, 2, 512], F32, tag="pp", bufs=3, name=name)

        def ps_tile(name, shape=(P, 512)):
            return pall.tile(list(shape), F32, tag="ps", bufs=2, name=name)

        k_slab = persist.tile([P, NT, P], BF16)      # k_rot^T: [d, ts_chunk, ts]
        v_slab = persist.tile([P, NT, P], BF16)      # v: [ts, ts_chunk, d]
        trim = persist.tile([P, P], BF16)            # 0/1 causal keep mask
        if not USE_GPSIMD_Z:
            ones = persist.tile([P, P], BF16)
            ones_f = persist.tile([P, 1], F32)
        c2_sb = persist.tile([P, T], BF16)
        s2_sb = persist.tile([P, T], BF16)
        wku_sb = persist.tile([P, LC, D], BF16)
        wvu_sb = persist.tile([P, LC, D], BF16)
        wku0_sb = persist.tile([P, LC, D], BF16)
        wvu0_sb = persist.tile([P, LC, D], BF16)
        wq_sb = persist.tile([P, KC, HG * P], BF16)
        wo_sb = persist.tile([P, HG, C], BF16)

        # --- init compute first (cheap; keeps engine queues clear up front)
        # 0/1 keep-mask: 1 where query(col) >= key(part), else 0
        nc.gpsimd.memset(trim[:], 1.0)
        nc.gpsimd.affine_select(
            out=trim[:], in_=trim[:],
            compare_op=mybir.AluOpType.is_ge,
            fill=0.0, base=0,
            pattern=[[1, P]], channel_multiplier=-1,
        )
        if not USE_GPSIMD_Z:
            nc.vector.memset(ones_f[:], 1.0)
            nc.vector.tensor_copy(ones[:], ones_f[:].to_broadcast([P, P]))

        # --- prologue DMAs, spread across the engine queues so the ~0.7us
        # per-dma_start issue overhead parallelizes and the first kv matmuls
        # start as soon as x(g0)+wkv(g0) land. wq slice 0 goes out early on
        # the tensor queue so qproj(h0) isn't starved (needed ~25us in).
        xts = [None] * NQ
        xts[0] = xpool.tile([P, KC, 512], BF16, tag="x", name="x0")
        wkv_sb = persist.tile([P, KC, L], BF16)
        WQS = HG * P // 4
        for g in range(8):
            nc.sync.dma_start(xts[0][:, bass.ts(g, 2), :],
                              xT_r[:, bass.ts(g, 2), bass.ts(0, 512)])
            nc.scalar.dma_start(wkv_sb[:, bass.ts(g, 2), :],
                                wkvT_r[:, bass.ts(g, 2), :])
        nc.gpsimd.dma_start(wq_sb[:, :, bass.ts(0, WQS)],
                            wqT_r[:, :, bass.ts(0, WQS)])
        nc.gpsimd.dma_start(
            wku0_sb[:], wku0T.rearrange("(lc p) d -> p lc d", p=P))
        nc.gpsimd.dma_start(
            wvu0_sb[:], wvu0T.rearrange("(lc p) d -> p lc d", p=P))
        nc.gpsimd.dma_start(c2_sb[:, 0:512], c2[:, 0:512])
        nc.gpsimd.dma_start(s2_sb[:, 0:512], s2[:, 0:512])
        nc.sync.dma_start(wq_sb[:, :, bass.ts(1, WQS)],
                          wqT_r[:, :, bass.ts(1, WQS)])
        nc.gpsimd.dma_start(wq_sb[:, :, bass.ts(2, WQS)],
                            wqT_r[:, :, bass.ts(2, WQS)])
        nc.sync.dma_start(wq_sb[:, :, bass.ts(3, WQS)],
                          wqT_r[:, :, bass.ts(3, WQS)])
        nc.gpsimd.dma_start(c2_sb[:, 512:T], c2[:, 512:T])
        nc.gpsimd.dma_start(s2_sb[:, 512:T], s2[:, 512:T])
        nc.gpsimd.dma_start(wku_sb[:], wkuT_r)
        nc.gpsimd.dma_start(wvu_sb[:], wvuT_r)
        nc.gpsimd.collective_compute(
            "AllGather", mybir.AluOpType.bypass,
            [[0, 1], [2, 3], [4, 5], [6, 7]], ins=[ccw_s], outs=[ccw_d])

        state = {"pending": None}
        qts = [None] * HG
        kvns = {}

        def rope_copy(ps):
            # single PSUM read via ACT so the bank frees immediately
            qq = rpool.tile([P, 512], BF16, tag="qq")
            nc.scalar.copy(qq[:], ps[:])
            return qq

        def rope_mults(dst, qq, n):
            # dst = qq * c2 + swap64(qq) * s2 on DVE (the two-input TT ops
            # require equal base partitions, so the swap stays a copy)
            c2n = c2_sb[:, bass.ts(n, 512)]
            s2n = s2_sb[:, bass.ts(n, 512)]
            qs = rpool.tile([P, 512], BF16, tag="qs")
            nc.vector.tensor_copy(qs[0:64, :], qq[64:128, :])
            nc.vector.tensor_copy(qs[64:128, :], qq[0:64, :])
            nc.vector.tensor_tensor(qs[:], qs[:], s2n, mybir.AluOpType.mult)
            nc.vector.tensor_tensor(qq[:], qq[:], c2n, mybir.AluOpType.mult)
            nc.vector.tensor_tensor(dst, qq[:], qs[:], mybir.AluOpType.add)

        def rope(dst, ps, n):
            rope_mults(dst, rope_copy(ps), n)

        def flush_zpv(staged=False):
            # staged=True returns (yp-matmul thunks, finalizer) so the PV
            # chain can be spread through the scores tail. The PV inputs are
            # two heads old - zero dep risk.
            if state["pending"] is None:
                return None
            n, h, nts, spans, exp_t, zgm, zsum, y_t = state["pending"]
            state["pending"] = None
            from collections import deque
            yp = ps_tile(f"yp{n}_{h}")

            def mk(j):
                def go():
                    sl = slice(spans[j], 512)
                    nc.tensor.matmul(yp[:, sl], v_slab[:, j, :],
                                     exp_t[:, j, sl],
                                     start=(j == 0), stop=(j == nts - 1))
                return go
            fills = deque(mk(j) for j in range(nts))

            def fin():
                zr = zpool.tile([P, 512], F32, tag="zr", bufs=1)
                if USE_GPSIMD_Z:
                    nc.vector.reciprocal_approx_fast(out=zr[:], in_=zsum[:])
                else:
                    zp = ps_tile(f"zp{n}_{h}")
                    nc.tensor.matmul(zp[:], ones[:], zgm[:], start=True,
                                     stop=True)
                    nc.vector.reciprocal_approx_fast(out=zr[:], in_=zp[:])
                nc.vector.tensor_tensor(y_t[:, h, :], yp[:], zr[:],
                                        mybir.AluOpType.mult)
            if staged:
                return fills, fin
            while fills:
                fills.popleft()()
            fin()
            return None

        def emit_qproj_mms(n, h):
            # q projection chain + the PSUM-freeing ACT copy; the DVE rope
            # multiplies are emitted separately at the end of the iteration so
            # they queue BEHIND the previous head's mask/z-adds on DVE
            qp = ps_tile(f"qp{n}_{h}")
            for kc in range(KC):
                nc.tensor.matmul(qp[:], wq_sb[:, kc, bass.ts(h, P)],
                                 xts[n][:, kc, :],
                                 start=(kc == 0), stop=(kc == KC - 1))
            return rope_copy(qp)

        def emit_rope_q(n, h, qq):
            qt = qtp.tile([P, 512], BF16, tag="qt", name=f"q{n}_{h}")
            rope_mults(qt[:], qq, n)
            qts[h] = qt

        def emit_scores(n, h, y_t):
            nts = 4 * n + 4
            npairs = nts // 2
            spans = [max(P * j - 512 * n, 0) for j in range(nts)]
            exp_t = epool.tile([P, NT, 512], BF16, tag="exp", name=f"e{n}_{h}")
            q_t = qts[h]
            # bf16 partial sums of exp over key blocks on DVE: unmasked pairs
            # as [P,2,512] flat ops, the 4 diagonal blocks into a separate
            # accumulator, folded at the end
            zg2 = zpool.tile([P, 2, 512], BF16, tag="zg2", name=f"zg2_{n}_{h}")
            zgm = zpool.tile([P, 512], BF16, tag="zgm", name=f"zgm{n}_{h}")
            flush_pair = 2 if nts > 4 else 0
            staged = None

            def pump(k):
                nonlocal staged
                if staged is not None:
                    pfills, pfin = staged
                    for _ in range(k):
                        if pfills:
                            pfills.popleft()()
                    if not pfills:
                        pfin()
                        staged = None

            for pi in range(npairs):
                if pi == flush_pair:
                    if flush_pair > 0:
                        staged = flush_zpv(staged=True)
                    else:
                        flush_zpv()
                pg = pp_tile(f"sc{n}_{h}_{pi}")
                for s in range(2):
                    j = 2 * pi + s
                    sl = slice(spans[j], 512)
                    nc.tensor.matmul(pg[:, s, sl], k_slab[:, j, :],
                                     q_t[:, sl], start=True, stop=True)
                    pump(2)
                # one exp per pair: reads both banks in one ACT op; columns
                # below the span hold garbage and are never read downstream
                nc.scalar.activation(
                    exp_t[:, 2 * pi:2 * pi + 2, :], pg[:],
                    mybir.ActivationFunctionType.Exp, scale=SCALE)
                if 2 * pi >= 4 * n:
                    # diagonal pair: causal mask as 0/1 multiply on exp output
                    for s in range(2):
                        j = 2 * pi + s
                        g = spans[j]
                        nc.vector.tensor_tensor(
                            exp_t[:, j, g:g + P], exp_t[:, j, g:g + P],
                            trim[:], mybir.AluOpType.mult)
                    for s in range(2):
                        j = 2 * pi + s
                        sl = slice(spans[j], 512)
                        if j == 4 * n:
                            nc.vector.tensor_copy(zgm[:], exp_t[:, j, :])
                        else:
                            nc.vector.tensor_tensor(zgm[:, sl], zgm[:, sl],
                                                    exp_t[:, j, sl],
                                                    mybir.AluOpType.add)
                else:
                    pair = exp_t[:, 2 * pi:2 * pi + 2, :]
                    if pi == 0:
                        nc.vector.tensor_copy(zg2[:], pair)
                    else:
                        nc.vector.tensor_tensor(zg2[:], zg2[:], pair,
                                                mybir.AluOpType.add)
                pump(2)
            if staged is not None:
                pfills, pfin = staged
                while pfills:
                    pfills.popleft()()
                pfin()
                staged = None
            if n > 0:
                nc.vector.tensor_tensor(zgm[:], zgm[:], zg2[:, 0, :],
                                        mybir.AluOpType.add)
                nc.vector.tensor_tensor(zgm[:], zgm[:], zg2[:, 1, :],
                                        mybir.AluOpType.add)
            if flush_pair == 0 and state["pending"] is not None:
                flush_zpv()
            if USE_GPSIMD_Z:
                zsum = zpool.tile([P, 512], F32, tag="zs", bufs=1,
                                  name=f"zs{n}_{h}")
                nc.gpsimd.partition_all_reduce(zsum[:], zgm[:], P,
                                               bass_isa.ReduceOp.add)
            else:
                zsum = None
            state["pending"] = (n, h, nts, spans, exp_t, zgm, zsum, y_t)

        kvps_st = {}

        def emit_kv_head(n, kcs):
            # chunk 0: full latent kv locally; chunks 1-3: only this core's
            # HALF (wkv columns are own-half-first per core) - the pair core
            # computes the other half and an AllGather merges them
            nl = LC if n == 0 else 2
            if n not in kvps_st:
                kvps_st[n] = [pp_tile(f"kv{n}_{i}") for i in range(nl // 2)]
            for kc in kcs:
                for lc in range(nl):
                    nc.tensor.matmul(kvps_st[n][lc // 2][:, lc % 2, :],
                                     wkv_sb[:, kc, bass.ts(lc, P)],
                                     xts[n][:, kc, :],
                                     start=(kc == 0), stop=(kc == KC - 1))

        def emit_kv_stage(n):
            kvps = kvps_st.pop(n)
            kvh = kvpool.tile([P, 2, 512], BF16, tag="kvh")
            nc.scalar.copy(kvh[:], kvps[0][:])
            nc.sync.dma_start(
                ccs[n - 1].rearrange("(s p) t -> p s t", p=P), kvh[:])
            nc.gpsimd.collective_compute(
                "AllGather", mybir.AluOpType.bypass,
                [[0, 1], [2, 3], [4, 5], [6, 7]],
                ins=[ccs[n - 1]], outs=[ccd[n - 1]])
            # preload the gathered latent now so kv_tail(n) has no DMA wait
            kvn = kvpool.tile([P, LC, 512], BF16, tag="kvn", name=f"kvn{n}")
            nc.scalar.dma_start(
                kvn[:], ccd[n - 1].rearrange("b (s p) t -> p (b s) t", p=P))
            kvns[n] = kvn

        def emit_kv_tail(n):
            if n == 0:
                kvn = kvpool.tile([P, LC, 512], BF16, tag="kvn", name="kvn0")
                kvps = kvps_st.pop(0)
                nc.scalar.copy(kvn[:, 0:2, :], kvps[0][:])
                nc.scalar.copy(kvn[:, 2:4, :], kvps[1][:])
                wku, wvu = wku0_sb, wvu0_sb
            else:
                kvn = kvns.pop(n)
                wku, wvu = wku_sb, wvu_sb

            kp = ps_tile(f"kp{n}")
            for lc in range(LC):
                nc.tensor.matmul(kp[:], wku[:, lc, :], kvn[:, lc, :],
                                 start=(lc == 0), stop=(lc == LC - 1))
            kdst = k_slab[:, 4 * n:4 * (n + 1), :].rearrange("p a b -> p (a b)")
            rope(kdst, kp, n)

            vp = ps_tile(f"vp{n}", shape=(P, 4, P))
            for i in range(4):
                for lc in range(LC):
                    nc.tensor.matmul(vp[:, i, :], kvn[:, lc, bass.ts(i, P)],
                                     wvu[:, lc, :],
                                     start=(lc == 0), stop=(lc == LC - 1))
            nc.scalar.copy(v_slab[:, 4 * n:4 * n + 4, :], vp[:])

        def emit_C(n, y_t):
            # h-inner with ci under it: the ci matmuls share the y-chunk
            # stationary; drains split DVE/ACT; output DMA split across two
            # queues per 128-token row block
            for t in range(4):
                oph = [pp_tile(f"op{n}_{t}_{q}") for q in range(2)]
                for h in range(HG):
                    for ci in range(4):
                        nc.tensor.matmul(oph[ci // 2][:, ci % 2, :],
                                         y_t[:, h, bass.ts(t, P)],
                                         wo_sb[:, h, bass.ts(ci, 512)],
                                         start=(h == 0), stop=(h == HG - 1))
                ost = opool.tile([P, 4, 512], BF16, tag="ost")
                nc.vector.tensor_copy(ost[:, 0:2, :], oph[0][:])
                nc.scalar.copy(ost[:, 2:4, :], oph[1][:])
                row = bass.ts(4 * n + t, P)
                nc.sync.dma_start(
                    outp[row, 0:1024],
                    ost[:, 0:2, :].rearrange("p a b -> p (a b)"))
                nc.scalar.dma_start(
                    outp[row, 1024:2048],
                    ost[:, 2:4, :].rearrange("p a b -> p (a b)"))

        emit_kv_head(0, range(KC))
        for n in range(NQ):
            if n == 0:
                for i in range(4):
                    nc.scalar.dma_start(wo_sb[:, :, bass.ts(i, C // 4)],
                                        woT_r[:, :, bass.ts(i, C // 4)])
            if n + 1 < NQ:
                xts[n + 1] = xpool.tile([P, KC, 512], BF16, tag="x",
                                        name=f"x{n + 1}")
                nc.sync.dma_start(xts[n + 1][:],
                                  xT_r[:, :, bass.ts(n + 1, 512)])
            y_t = ypool.tile([P, HG, 512], BF16, tag="yc", name=f"y{n}")
            for h in range(HG):
                if h == 0:
                    emit_kv_tail(n)
                qq = emit_qproj_mms(n, h)
                if h >= 1:
                    emit_scores(n, h - 1, y_t)
                emit_rope_q(n, h, qq)
            # tail: first half of kv(n+1) covers the rope(h7) latency before
            # scores(h7); the stage + AllGather launch right after, hiding
            # the collective under the flush, emit_C and the next q chains
            if n + 1 < NQ:
                emit_kv_head(n + 1, range(0, KC // 2))
                emit_scores(n, HG - 1, y_t)
                emit_kv_head(n + 1, range(KC // 2, KC))
                emit_kv_stage(n + 1)
                flush_zpv()
            else:
                emit_scores(n, HG - 1, y_t)
                flush_zpv()
            emit_C(n, y_t)

    nc.finalize()
    return nc


_PERM = np.concatenate([np.arange(0, D, 2), np.arange(1, D, 2)])


def _prep_core_inputs(x, freqs_cos, freqs_sin, wq, wkv_down, wk_up, wv_up, wo):
    cosT = np.ascontiguousarray(freqs_cos.T)                      # [64, T]
    sinT = np.ascontiguousarray(freqs_sin.T)
    c2 = np.concatenate([cosT, cosT], axis=0).astype(NPBF16)      # [128, T]
    s2 = np.concatenate([-sinT, sinT], axis=0).astype(NPBF16)

    wkvT = np.ascontiguousarray(wkv_down.T).astype(NPBF16)        # [C, L]
    wkuT = np.ascontiguousarray(wk_up[_PERM, :].T).astype(NPBF16)  # [L, D]
    wvuT = np.ascontiguousarray(wv_up.T).astype(NPBF16)           # [L, D]

    wq_h = wq.reshape(H, D, C)[:, _PERM, :]                       # perm rows/head

    in_maps = []
    for core in range(N_CORES):
        b, g = core // 2, core % 2
        heads = slice(8 * g, 8 * g + 8)
        wqT_g = np.ascontiguousarray(
            wq_h[heads].reshape(HG * D, C).T).astype(NPBF16)      # [C, 1024]
        wkvT_g = np.ascontiguousarray(
            np.concatenate([wkvT[:, 256 * g:256 * g + 256],
                            wkvT[:, 256 * (1 - g):256 * (1 - g) + 256]],
                           axis=1))
        wku0T_g = np.ascontiguousarray(np.roll(wkuT, -256 * g, axis=0))
        wvu0T_g = np.ascontiguousarray(np.roll(wvuT, -256 * g, axis=0))
        woT_g = np.ascontiguousarray(
            wo[:, 8 * g * D:(8 * g + 8) * D].T).astype(NPBF16)    # [1024, C]
        xT_b = np.ascontiguousarray(x[b].T).astype(NPBF16)        # [C, T]
        in_maps.append({
            "xT": xT_b, "wqT": wqT_g, "wkvT": wkvT_g, "wkuT": wkuT,
            "wvuT": wvuT, "wku0T": wku0T_g, "wvu0T": wvu0T_g,
            "woT": woT_g, "c2": c2, "s2": s2,
        })
    return in_maps


def kernel(x, freqs_cos, freqs_sin, wq, wkv_down, wk_up, wv_up, wo, _trace=False):
    x = np.asarray(x, dtype=np.float32)
    freqs_cos = np.asarray(freqs_cos, dtype=np.float32)
    freqs_sin = np.asarray(freqs_sin, dtype=np.float32)
    wq = np.asarray(wq, dtype=np.float32)
    wkv_down = np.asarray(wkv_down, dtype=np.float32)
    wk_up = np.asarray(wk_up, dtype=np.float32)
    wv_up = np.asarray(wv_up, dtype=np.float32)
    wo = np.asarray(wo, dtype=np.float32)

    if "nc" not in _cached:
        _cached["nc"] = _build_program()
    nc = _cached["nc"]

    in_maps = _prep_core_inputs(x, freqs_cos, freqs_sin, wq, wkv_down,
                                wk_up, wv_up, wo)
    res = run_bass_kernel_spmd(nc, in_maps, core_ids=list(range(N_CORES)),
                               trace=_trace)
    _cached["last_result"] = res

    out = np.empty((B, T, C), dtype=np.float32)
    for b in range(B):
        out[b] = (res.results[2 * b]["outp"].astype(np.float32)
                  + res.results[2 * b + 1]["outp"].astype(np.float32))
    return out


# revision 13
# speedup vs baseline: 1.1385x; 1.1385x over previous
"""MLA (multi-head latent) causal attention on 8 Trainium2 NeuronCores.

Sharding: batch(4) x head-group(2) mesh over 8 cores. Core c handles batch
c//2 and heads [8*(c%2), 8*(c%2)+8). The latent KV projections are small and
recomputed per head-group (an MLA property: the latent KV is shared across
heads). Each core produces a partial output (its head-group's contribution to
y @ wo^T for its batch); the host sums the two partials per batch.

All data is bf16 (rel tolerance 2e-2 leaves plenty of room): this enables the
PE's Fast Weight Load path, halves HBM traffic, and doubles DVE throughput.
PSUM accumulation stays fp32.

Single fused loop over the 4 query chunks of 512 tokens; everything streams
through SBUF (no DRAM scratch):
  per chunk n:
    h-loop (software pipelined): q-proj(h) chain -> RoPE(h) on ACT+DVE,
      scores(h-1) as PAIRS of key blocks into 2-bank PSUM tiles -> one exp
      per pair on ACT -> causal mask as a 0/1 bf16 multiply on the exp
      output (DVE) -> bf16 partial sums of exp on DVE, PV interleaved
      mid-scores, softmax denominator via GPSIMD partition_all_reduce
      (keeps the PE free), 1/z normalize on DVE.
    tail: latent kv proj for chunk n+1 split around the AllGather stage,
      then the output projection with DVE+ACT evacuation in parallel and
      the output DMA split across two queues.

PSUM budget (8 banks): tag "pp" = 3 x [P,2,512] pairs (6 banks, score pairs /
latent halves / out-proj pairs), tag "ps" = 2 x [P,512] singles (q-proj, PV
accumulator, k-up, v-up).
"""

import math
from contextlib import ExitStack

import numpy as np

import concourse.bass as bass
import concourse.mybir as mybir
import concourse.tile as tile
from concourse import bacc
from concourse import bass_isa
from concourse.bass_utils import run_bass_kernel_spmd

# Problem shape (hardcoded per contract).
B, T, C = 4, 2048, 2048
H, D, L = 16, 128, 512
HG = 8           # heads per core
N_CORES = 8
P = 128
KC = C // P      # 16 contraction chunks over C
LC = L // P      # 4 chunks over L
NQ = T // 512    # 4 query chunks of 512
NT = T // P      # 16 key chunks of 128
SCALE = 1.0 / math.sqrt(D)

F32 = mybir.dt.float32
BF16 = mybir.dt.bfloat16
NPBF16 = mybir.dt.np(BF16)

USE_GPSIMD_Z = False  # gpsimd z trips the P0 power throttle (chip drops to 2.0GHz)

_cached = {}


def _build_program():
    nc = bacc.Bacc()

    xT = nc.dram_tensor("xT", [C, T], BF16, kind="ExternalInput").ap()
    wqT = nc.dram_tensor("wqT", [C, HG * D], BF16, kind="ExternalInput").ap()
    wkvT = nc.dram_tensor("wkvT", [C, L], BF16, kind="ExternalInput").ap()
    wkuT = nc.dram_tensor("wkuT", [L, D], BF16, kind="ExternalInput").ap()
    wvuT = nc.dram_tensor("wvuT", [L, D], BF16, kind="ExternalInput").ap()
    woT = nc.dram_tensor("woT", [HG * D, C], BF16, kind="ExternalInput").ap()
    c2 = nc.dram_tensor("c2", [P, T], BF16, kind="ExternalInput").ap()
    s2 = nc.dram_tensor("s2", [P, T], BF16, kind="ExternalInput").ap()
    outp = nc.dram_tensor("outp", [T, C], BF16, kind="ExternalOutput").ap()
    wku0T = nc.dram_tensor("wku0T", [L, D], BF16, kind="ExternalInput").ap()
    wvu0T = nc.dram_tensor("wvu0T", [L, D], BF16, kind="ExternalInput").ap()
    ccw_s = nc.dram_tensor("ccw_s", [P, 16], BF16, kind="Internal").ap()
    ccw_d = nc.dram_tensor("ccw_d", [2, P, 16], BF16, kind="Internal").ap()
    ccs = [nc.dram_tensor(f"ccs{i}", [L // 2, 512], BF16, kind="Internal").ap()
           for i in range(1, NQ)]
    ccd = [nc.dram_tensor(f"ccd{i}", [2, L // 2, 512], BF16,
                          kind="Internal").ap() for i in range(1, NQ)]

    xT_r = xT.rearrange("(kc p) t -> p kc t", p=P)
    wqT_r = wqT.rearrange("(kc p) e -> p kc e", p=P)
    wkvT_r = wkvT.rearrange("(kc p) l -> p kc l", p=P)
    wkuT_r = wkuT.rearrange("(lc p) d -> p lc d", p=P)
    wvuT_r = wvuT.rearrange("(lc p) d -> p lc d", p=P)
    woT_r = woT.rearrange("(h p) c -> p h c", p=P)

    with tile.TileContext(nc) as tc, ExitStack() as top:
        persist = top.enter_context(tc.tile_pool(name="persist", bufs=1))
        pall = top.enter_context(tc.tile_pool(name="pall", bufs=1, space="PSUM"))
        xpool = top.enter_context(tc.tile_pool(name="xpool", bufs=2))
        kvpool = top.enter_context(tc.tile_pool(name="kvpool", bufs=1))
        rpool = top.enter_context(tc.tile_pool(name="rpool", bufs=2))
        qtp = top.enter_context(tc.tile_pool(name="qtp", bufs=3))
        epool = top.enter_context(tc.tile_pool(name="epool", bufs=2))
        zpool = top.enter_context(tc.tile_pool(name="zpool", bufs=2))
        ypool = top.enter_context(tc.tile_pool(name="ypool", bufs=2))
        opool = top.enter_context(tc.tile_pool(name="opool", bufs=1))

        def pp_tile(name):
            return pall.tile([P, 2, 512], F32, tag="pp", bufs=3, name=name)

        def ps_tile(name, shape=(P, 512)):
            return pall.tile(list(shape), F32, tag="ps", bufs=2, name=name)

        k_slab = persist.tile([P, NT, P], BF16)      # k_rot^T: [d, ts_chunk, ts]
        v_slab = persist.tile([P, NT, P], BF16)      # v: [ts, ts_chunk, d]
        trim = persist.tile([P, P], BF16)            # 0/1 causal keep mask
        if not USE_GPSIMD_Z:
            ones = persist.tile([P, P], BF16)
            ones_f = persist.tile([P, 1], F32)
        c2_sb = persist.tile([P, T], BF16)
        s2_sb = persist.tile([P, T], BF16)
        wku_sb = persist.tile([P, LC, D], BF16)
        wvu_sb = persist.tile([P, LC, D], BF16)
        wku0_sb = persist.tile([P, LC, D], BF16)
        wvu0_sb = persist.tile([P, LC, D], BF16)
        wq_sb = persist.tile([P, KC, HG * P], BF16)
        wo_sb = persist.tile([P, HG, C], BF16)

        # --- init compute first (cheap; keeps engine queues clear up front)
        # 0/1 keep-mask: 1 where query(col) >= key(part), else 0
        nc.gpsimd.memset(trim[:], 1.0)
        nc.gpsimd.affine_select(
            out=trim[:], in_=trim[:],
            compare_op=mybir.AluOpType.is_ge,
            fill=0.0, base=0,
            pattern=[[1, P]], channel_multiplier=-1,
        )
        if not USE_GPSIMD_Z:
            nc.vector.memset(ones_f[:], 1.0)
            nc.vector.tensor_copy(ones[:], ones_f[:].to_broadcast([P, P]))

        # --- prologue DMAs, spread across the engine queues so the ~0.7us
        # per-dma_start issue overhead parallelizes and the first kv matmuls
        # start as soon as x(g0)+wkv(g0) land. wq slice 0 goes out early on
        # the tensor queue so qproj(h0) isn't starved (needed ~25us in).
        xts = [None] * NQ
        xts[0] = xpool.tile([P, KC, 512], BF16, tag="x", name="x0")
        wkv_sb = persist.tile([P, KC, L], BF16)
        # wq in head-sized column slices interleaved with x0 on the sync
        # queue: slice k arrives just before head k's qproj needs it
        wq_slices = [(0, 128), (128, 256), (256, 512), (512, 768),
                     (768, 1024)]
        wq_after = {1: 0, 3: 1, 5: 2, 7: 3}  # x-group -> wq slice idx
        for g in range(8):
            nc.sync.dma_start(xts[0][:, bass.ts(g, 2), :],
                              xT_r[:, bass.ts(g, 2), bass.ts(0, 512)])
            nc.scalar.dma_start(wkv_sb[:, bass.ts(g, 2), :],
                                wkvT_r[:, bass.ts(g, 2), :])
            if g in wq_after:
                a, b = wq_slices[wq_after[g]]
                nc.sync.dma_start(wq_sb[:, :, a:b], wqT_r[:, :, a:b])
        a, b = wq_slices[4]
        nc.sync.dma_start(wq_sb[:, :, a:b], wqT_r[:, :, a:b])
        nc.gpsimd.dma_start(c2_sb[:, 0:512], c2[:, 0:512])
        nc.gpsimd.dma_start(s2_sb[:, 0:512], s2[:, 0:512])
        nc.gpsimd.dma_start(
            wku0_sb[:], wku0T.rearrange("(lc p) d -> p lc d", p=P))
        nc.gpsimd.dma_start(
            wvu0_sb[:], wvu0T.rearrange("(lc p) d -> p lc d", p=P))
        nc.gpsimd.dma_start(c2_sb[:, 512:T], c2[:, 512:T])
        nc.gpsimd.dma_start(s2_sb[:, 512:T], s2[:, 512:T])
        nc.gpsimd.dma_start(wku_sb[:], wkuT_r)
        nc.gpsimd.dma_start(wvu_sb[:], wvuT_r)
        nc.gpsimd.collective_compute(
            "AllGather", mybir.AluOpType.bypass,
            [[0, 1], [2, 3], [4, 5], [6, 7]], ins=[ccw_s], outs=[ccw_d])

        state = {"pending": None}
        qts = [None] * HG
        kvns = {}

        def rope_copy(ps):
            # single PSUM read via ACT so the bank frees immediately
            qq = rpool.tile([P, 512], BF16, tag="qq")
            nc.scalar.copy(qq[:], ps[:])
            return qq

        def rope_mults(dst, qq, n):
            # dst = qq * c2 + swap64(qq) * s2 on DVE (the two-input TT ops
            # require equal base partitions, so the swap stays a copy)
            c2n = c2_sb[:, bass.ts(n, 512)]
            s2n = s2_sb[:, bass.ts(n, 512)]
            qs = rpool.tile([P, 512], BF16, tag="qs")
            nc.vector.tensor_copy(qs[0:64, :], qq[64:128, :])
            nc.vector.tensor_copy(qs[64:128, :], qq[0:64, :])
            nc.vector.tensor_tensor(qs[:], qs[:], s2n, mybir.AluOpType.mult)
            nc.vector.tensor_tensor(qq[:], qq[:], c2n, mybir.AluOpType.mult)
            nc.vector.tensor_tensor(dst, qq[:], qs[:], mybir.AluOpType.add)

        def rope(dst, ps, n):
            rope_mults(dst, rope_copy(ps), n)

        def flush_zpv(staged=False):
            # staged=True returns (yp-matmul thunks, finalizer) so the PV
            # chain can be spread through the scores tail. The PV inputs are
            # two heads old - zero dep risk.
            if state["pending"] is None:
                return None
            n, h, nts, spans, exp_t, zgm, zsum, y_t = state["pending"]
            state["pending"] = None
            from collections import deque
            yp = ps_tile(f"yp{n}_{h}")

            def mk(j):
                def go():
                    sl = slice(spans[j], 512)
                    nc.tensor.matmul(yp[:, sl], v_slab[:, j, :],
                                     exp_t[:, j, sl],
                                     start=(j == 0), stop=(j == nts - 1))
                return go
            fills = deque(mk(j) for j in range(nts))

            def fin():
                zr = zpool.tile([P, 512], F32, tag="zr", bufs=1)
                if USE_GPSIMD_Z:
                    nc.vector.reciprocal_approx_fast(out=zr[:], in_=zsum[:])
                else:
                    zp = ps_tile(f"zp{n}_{h}")
                    nc.tensor.matmul(zp[:], ones[:], zgm[:], start=True,
                                     stop=True)
                    nc.vector.reciprocal_approx_fast(out=zr[:], in_=zp[:])
                nc.vector.tensor_tensor(y_t[:, h, :], yp[:], zr[:],
                                        mybir.AluOpType.mult)
            if staged:
                return fills, fin
            while fills:
                fills.popleft()()
            fin()
            return None

        def emit_qproj_mms(n, h):
            # q projection chain + the PSUM-freeing ACT copy; the DVE rope
            # multiplies are emitted separately at the end of the iteration so
            # they queue BEHIND the previous head's mask/z-adds on DVE
            qp = ps_tile(f"qp{n}_{h}")
            for kc in range(KC):
                nc.tensor.matmul(qp[:], wq_sb[:, kc, bass.ts(h, P)],
                                 xts[n][:, kc, :],
                                 start=(kc == 0), stop=(kc == KC - 1))
            return rope_copy(qp)

        def emit_rope_q(n, h, qq):
            qt = qtp.tile([P, 512], BF16, tag="qt", name=f"q{n}_{h}")
            rope_mults(qt[:], qq, n)
            qts[h] = qt

        def emit_scores(n, h, y_t):
            nts = 4 * n + 4
            npairs = nts // 2
            spans = [max(P * j - 512 * n, 0) for j in range(nts)]
            exp_t = epool.tile([P, NT, 512], BF16, tag="exp", name=f"e{n}_{h}")
            q_t = qts[h]
            # bf16 partial sums of exp over key blocks on DVE: unmasked pairs
            # as [P,2,512] flat ops, the 4 diagonal blocks into a separate
            # accumulator, folded at the end
            zg2 = zpool.tile([P, 2, 512], BF16, tag="zg2", name=f"zg2_{n}_{h}")
            zgm = zpool.tile([P, 512], BF16, tag="zgm", name=f"zgm{n}_{h}")
            flush_pair = 2 if nts > 4 else 0
            staged = None

            def pump(k):
                nonlocal staged
                if staged is not None:
                    pfills, pfin = staged
                    for _ in range(k):
                        if pfills:
                            pfills.popleft()()
                    if not pfills:
                        # defer the reciprocal+normalize so it lands BEHIND
                        # this head's rope multiplies in the DVE FIFO (its
                        # z input is gpsimd-produced, two heads old)
                        state["late_fin"] = pfin
                        staged = None

            for pi in range(npairs):
                if pi == flush_pair:
                    if flush_pair > 0:
                        staged = flush_zpv(staged=True)
                    else:
                        flush_zpv()
                pg = pp_tile(f"sc{n}_{h}_{pi}")
                for s in range(2):
                    j = 2 * pi + s
                    sl = slice(spans[j], 512)
                    nc.tensor.matmul(pg[:, s, sl], k_slab[:, j, :],
                                     q_t[:, sl], start=True, stop=True)
                    pump(2)
                # one exp per pair, exact-width from the first block's span:
                # columns below it hold garbage and are never read downstream
                g0 = spans[2 * pi]
                nc.scalar.activation(
                    exp_t[:, 2 * pi:2 * pi + 2, g0:512], pg[:, :, g0:512],
                    mybir.ActivationFunctionType.Exp, scale=SCALE)
                if 2 * pi >= 4 * n:
                    # diagonal pair: causal mask as 0/1 multiply on exp output
                    for s in range(2):
                        j = 2 * pi + s
                        g = spans[j]
                        nc.vector.tensor_tensor(
                            exp_t[:, j, g:g + P], exp_t[:, j, g:g + P],
                            trim[:], mybir.AluOpType.mult)
                    for s in range(2):
                        j = 2 * pi + s
                        sl = slice(spans[j], 512)
                        if j == 4 * n:
                            nc.vector.tensor_copy(zgm[:], exp_t[:, j, :])
                        else:
                            nc.vector.tensor_tensor(zgm[:, sl], zgm[:, sl],
                                                    exp_t[:, j, sl],
                                                    mybir.AluOpType.add)
                else:
                    pair = exp_t[:, 2 * pi:2 * pi + 2, :]
                    if pi == 0:
                        nc.vector.tensor_copy(zg2[:], pair)
                    else:
                        nc.vector.tensor_tensor(zg2[:], zg2[:], pair,
                                                mybir.AluOpType.add)
                pump(2)
            if staged is not None:
                pfills, pfin = staged
                while pfills:
                    pfills.popleft()()
                state["late_fin"] = pfin
                staged = None
            if n > 0:
                nc.vector.tensor_tensor(zgm[:], zgm[:], zg2[:, 0, :],
                                        mybir.AluOpType.add)
                nc.vector.tensor_tensor(zgm[:], zgm[:], zg2[:, 1, :],
                                        mybir.AluOpType.add)
            if flush_pair == 0 and state["pending"] is not None:
                flush_zpv()
            if USE_GPSIMD_Z:
                zsum = zpool.tile([P, 512], F32, tag="zs", bufs=1,
                                  name=f"zs{n}_{h}")
                nc.gpsimd.partition_all_reduce(zsum[:], zgm[:], P,
                                               bass_isa.ReduceOp.add)
            else:
                zsum = None
            state["pending"] = (n, h, nts, spans, exp_t, zgm, zsum, y_t)

        kvps_st = {}

        def emit_kv_head(n, kcs):
            # chunk 0: full latent kv locally; chunks 1-3: only this core's
            # HALF (wkv columns are own-half-first per core) - the pair core
            # computes the other half and an AllGather merges them
            nl = LC if n == 0 else 2
            if n not in kvps_st:
                kvps_st[n] = [pp_tile(f"kv{n}_{i}") for i in range(nl // 2)]
            for kc in kcs:
                for lc in range(nl):
                    nc.tensor.matmul(kvps_st[n][lc // 2][:, lc % 2, :],
                                     wkv_sb[:, kc, bass.ts(lc, P)],
                                     xts[n][:, kc, :],
                                     start=(kc == 0), stop=(kc == KC - 1))

        def emit_kv_stage(n):
            kvps = kvps_st.pop(n)
            kvh = kvpool.tile([P, 2, 512], BF16, tag="kvh")
            nc.scalar.copy(kvh[:], kvps[0][:])
            nc.sync.dma_start(
                ccs[n - 1].rearrange("(s p) t -> p s t", p=P), kvh[:])
            nc.gpsimd.collective_compute(
                "AllGather", mybir.AluOpType.bypass,
                [[0, 1], [2, 3], [4, 5], [6, 7]],
                ins=[ccs[n - 1]], outs=[ccd[n - 1]])
            # preload the gathered latent now so kv_tail(n) has no DMA wait
            kvn = kvpool.tile([P, LC, 512], BF16, tag="kvn", name=f"kvn{n}")
            nc.scalar.dma_start(
                kvn[:], ccd[n - 1].rearrange("b (s p) t -> p (b s) t", p=P))
            kvns[n] = kvn

        def emit_kv_tail(n):
            if n == 0:
                kvn = kvpool.tile([P, LC, 512], BF16, tag="kvn", name="kvn0")
                kvps = kvps_st.pop(0)
                nc.scalar.copy(kvn[:, 0:2, :], kvps[0][:])
                nc.scalar.copy(kvn[:, 2:4, :], kvps[1][:])
                wku, wvu = wku0_sb, wvu0_sb
            else:
                kvn = kvns.pop(n)
                wku, wvu = wku_sb, wvu_sb

            kp = ps_tile(f"kp{n}")
            for lc in range(LC):
                nc.tensor.matmul(kp[:], wku[:, lc, :], kvn[:, lc, :],
                                 start=(lc == 0), stop=(lc == LC - 1))
            kdst = k_slab[:, 4 * n:4 * (n + 1), :].rearrange("p a b -> p (a b)")
            rope(kdst, kp, n)

            vp = ps_tile(f"vp{n}", shape=(P, 4, P))
            for i in range(4):
                for lc in range(LC):
                    nc.tensor.matmul(vp[:, i, :], kvn[:, lc, bass.ts(i, P)],
                                     wvu[:, lc, :],
                                     start=(lc == 0), stop=(lc == LC - 1))
            nc.scalar.copy(v_slab[:, 4 * n:4 * n + 4, :], vp[:])

        def emit_C(n, y_t):
            # h-inner with ci under it: the ci matmuls share the y-chunk
            # stationary; drains split DVE/ACT; output DMA split across two
            # queues per 128-token row block
            for t in range(4):
                oph = [pp_tile(f"op{n}_{t}_{q}") for q in range(2)]
                for h in range(HG):
                    for ci in range(4):
                        nc.tensor.matmul(oph[ci // 2][:, ci % 2, :],
                                         y_t[:, h, bass.ts(t, P)],
                                         wo_sb[:, h, bass.ts(ci, 512)],
                                         start=(h == 0), stop=(h == HG - 1))
                ost = opool.tile([P, 4, 512], BF16, tag="ost")
                nc.vector.tensor_copy(ost[:, 0:2, :], oph[0][:])
                nc.scalar.copy(ost[:, 2:4, :], oph[1][:])
                row = bass.ts(4 * n + t, P)
                nc.sync.dma_start(
                    outp[row, 0:1024],
                    ost[:, 0:2, :].rearrange("p a b -> p (a b)"))
                nc.scalar.dma_start(
                    outp[row, 1024:2048],
                    ost[:, 2:4, :].rearrange("p a b -> p (a b)"))

        def run_late_fin():
            f = state.pop("late_fin", None)
            if f is not None:
                f()

        emit_kv_head(0, range(KC))
        for n in range(NQ):
            y_t = ypool.tile([P, HG, 512], BF16, tag="yc", name=f"y{n}")
            for h in range(HG):
                if h == 0:
                    emit_kv_tail(n)
                qq = emit_qproj_mms(n, h)
                # rope right after qproj: its DVE ops can start the moment
                # the ACT copy lands, ahead of this iteration's zg adds and
                # the deferred fin - shortest path to the next scores MMs
                emit_rope_q(n, h, qq)
                if h >= 1:
                    emit_scores(n, h - 1, y_t)
                    run_late_fin()
                if h == 2 and n + 1 < NQ:
                    xts[n + 1] = xpool.tile([P, KC, 512], BF16, tag="x",
                                            name=f"x{n + 1}")
                    nc.sync.dma_start(xts[n + 1][:],
                                      xT_r[:, :, bass.ts(n + 1, 512)])
                if h == 4 and n == 0:
                    for i in range(4):
                        nc.sync.dma_start(wo_sb[:, :, bass.ts(i, C // 4)],
                                          woT_r[:, :, bass.ts(i, C // 4)])
            # tail: first half of kv(n+1) covers the rope(h7) latency before
            # scores(h7); the stage + AllGather launch right after, hiding
            # the collective under the flush, emit_C and the next q chains
            if n + 1 < NQ:
                emit_kv_head(n + 1, range(0, KC // 2))
                emit_scores(n, HG - 1, y_t)
                run_late_fin()
                emit_kv_head(n + 1, range(KC // 2, KC))
                emit_kv_stage(n + 1)
                flush_zpv()
            else:
                emit_scores(n, HG - 1, y_t)
                run_late_fin()
                flush_zpv()
            emit_C(n, y_t)

    nc.finalize()
    return nc


_PERM = np.concatenate([np.arange(0, D, 2), np.arange(1, D, 2)])


def _prep_core_inputs(x, freqs_cos, freqs_sin, wq, wkv_down, wk_up, wv_up, wo):
    cosT = np.ascontiguousarray(freqs_cos.T)                      # [64, T]
    sinT = np.ascontiguousarray(freqs_sin.T)
    c2 = np.concatenate([cosT, cosT], axis=0).astype(NPBF16)      # [128, T]
    s2 = np.concatenate([-sinT, sinT], axis=0).astype(NPBF16)

    wkvT = np.ascontiguousarray(wkv_down.T).astype(NPBF16)        # [C, L]
    wkuT = np.ascontiguousarray(wk_up[_PERM, :].T).astype(NPBF16)  # [L, D]
    wvuT = np.ascontiguousarray(wv_up.T).astype(NPBF16)           # [L, D]

    wq_h = wq.reshape(H, D, C)[:, _PERM, :]                       # perm rows/head

    in_maps = []
    for core in range(N_CORES):
        b, g = core // 2, core % 2
        heads = slice(8 * g, 8 * g + 8)
        wqT_g = np.ascontiguousarray(
            wq_h[heads].reshape(HG * D, C).T).astype(NPBF16)      # [C, 1024]
        wkvT_g = np.ascontiguousarray(
            np.concatenate([wkvT[:, 256 * g:256 * g + 256],
                            wkvT[:, 256 * (1 - g):256 * (1 - g) + 256]],
                           axis=1))
        wku0T_g = np.ascontiguousarray(np.roll(wkuT, -256 * g, axis=0))
        wvu0T_g = np.ascontiguousarray(np.roll(wvuT, -256 * g, axis=0))
        woT_g = np.ascontiguousarray(
            wo[:, 8 * g * D:(8 * g + 8) * D].T).astype(NPBF16)    # [1024, C]
        xT_b = np.ascontiguousarray(x[b].T).astype(NPBF16)        # [C, T]
        in_maps.append({
            "xT": xT_b, "wqT": wqT_g, "wkvT": wkvT_g, "wkuT": wkuT,
            "wvuT": wvuT, "wku0T": wku0T_g, "wvu0T": wvu0T_g,
            "woT": woT_g, "c2": c2, "s2": s2,
        })
    return in_maps


def kernel(x, freqs_cos, freqs_sin, wq, wkv_down, wk_up, wv_up, wo, _trace=False):
    x = np.asarray(x, dtype=np.float32)
    freqs_cos = np.asarray(freqs_cos, dtype=np.float32)
    freqs_sin = np.asarray(freqs_sin, dtype=np.float32)
    wq = np.asarray(wq, dtype=np.float32)
    wkv_down = np.asarray(wkv_down, dtype=np.float32)
    wk_up = np.asarray(wk_up, dtype=np.float32)
    wv_up = np.asarray(wv_up, dtype=np.float32)
    wo = np.asarray(wo, dtype=np.float32)

    if "nc" not in _cached:
        _cached["nc"] = _build_program()
    nc = _cached["nc"]

    in_maps = _prep_core_inputs(x, freqs_cos, freqs_sin, wq, wkv_down,
                                wk_up, wv_up, wo)
    res = run_bass_kernel_spmd(nc, in_maps, core_ids=list(range(N_CORES)),
                               trace=_trace)
    _cached["last_result"] = res

    out = np.empty((B, T, C), dtype=np.float32)
    for b in range(B):
        out[b] = (res.results[2 * b]["outp"].astype(np.float32)
                  + res.results[2 * b + 1]["outp"].astype(np.float32))
    return out


# revision 17
# speedup vs baseline: 1.1793x; 1.0358x over previous
"""MLA (multi-head latent) causal attention on 8 Trainium2 NeuronCores.

Sharding: batch(4) x head-group(2) mesh over 8 cores. Core c handles batch
c//2 and heads [8*(c%2), 8*(c%2)+8). The latent KV projections are small and
recomputed per head-group (an MLA property: the latent KV is shared across
heads). Each core produces a partial output (its head-group's contribution to
y @ wo^T for its batch); the host sums the two partials per batch.

All data is bf16 (rel tolerance 2e-2 leaves plenty of room): this enables the
PE's Fast Weight Load path, halves HBM traffic, and doubles DVE throughput.
PSUM accumulation stays fp32.

Single fused loop over the 4 query chunks of 512 tokens; everything streams
through SBUF (no DRAM scratch):
  per chunk n:
    h-loop (software pipelined): q-proj(h) chain -> RoPE(h) on ACT+DVE,
      scores(h-1) as PAIRS of key blocks into 2-bank PSUM tiles -> one exp
      per pair on ACT -> causal mask as a 0/1 bf16 multiply on the exp
      output (DVE) -> bf16 partial sums of exp on DVE, PV interleaved
      mid-scores, softmax denominator via GPSIMD partition_all_reduce
      (keeps the PE free), 1/z normalize on DVE.
    tail: latent kv proj for chunk n+1 split around the AllGather stage,
      then the output projection with DVE+ACT evacuation in parallel and
      the output DMA split across two queues.

PSUM budget (8 banks): tag "pp" = 3 x [P,2,512] pairs (6 banks, score pairs /
latent halves / out-proj pairs), tag "ps" = 2 x [P,512] singles (q-proj, PV
accumulator, k-up, v-up).
"""

import math
from contextlib import ExitStack

import numpy as np

import concourse.bass as bass
import concourse.mybir as mybir
import concourse.tile as tile
from concourse import bacc
from concourse import bass_isa
from concourse.bass_utils import run_bass_kernel_spmd

# Problem shape (hardcoded per contract).
B, T, C = 4, 2048, 2048
H, D, L = 16, 128, 512
HG = 8           # heads per core
N_CORES = 8
P = 128
KC = C // P      # 16 contraction chunks over C
LC = L // P      # 4 chunks over L
NQ = T // 512    # 4 query chunks of 512
NT = T // P      # 16 key chunks of 128
SCALE = 1.0 / math.sqrt(D)

F32 = mybir.dt.float32
BF16 = mybir.dt.bfloat16
NPBF16 = mybir.dt.np(BF16)

USE_GPSIMD_Z = False  # gpsimd z trips the P0 power throttle (chip drops to 2.0GHz)

_cached = {}


def _build_program():
    nc = bacc.Bacc()

    xT = nc.dram_tensor("xT", [C, T], BF16, kind="ExternalInput").ap()
    wqT = nc.dram_tensor("wqT", [C, HG * D], BF16, kind="ExternalInput").ap()
    wkvT = nc.dram_tensor("wkvT", [C, L], BF16, kind="ExternalInput").ap()
    wkuT = nc.dram_tensor("wkuT", [L, D], BF16, kind="ExternalInput").ap()
    wvuT = nc.dram_tensor("wvuT", [L, D], BF16, kind="ExternalInput").ap()
    woT = nc.dram_tensor("woT", [HG * D, C], BF16, kind="ExternalInput").ap()
    c2 = nc.dram_tensor("c2", [P, T], BF16, kind="ExternalInput").ap()
    s2 = nc.dram_tensor("s2", [P, T], BF16, kind="ExternalInput").ap()
    outp = nc.dram_tensor("outp", [T, C], BF16, kind="ExternalOutput").ap()
    wku0T = nc.dram_tensor("wku0T", [L, D], BF16, kind="ExternalInput").ap()
    wvu0T = nc.dram_tensor("wvu0T", [L, D], BF16, kind="ExternalInput").ap()
    ccw_s = nc.dram_tensor("ccw_s", [P, 16], BF16, kind="Internal").ap()
    ccw_d = nc.dram_tensor("ccw_d", [2, P, 16], BF16, kind="Internal").ap()
    ccs = [nc.dram_tensor(f"ccs{i}", [L // 2, 512], BF16, kind="Internal").ap()
           for i in range(1, NQ)]
    ccd = [nc.dram_tensor(f"ccd{i}", [2, L // 2, 512], BF16,
                          kind="Internal").ap() for i in range(1, NQ)]

    xT_r = xT.rearrange("(kc p) t -> p kc t", p=P)
    wqT_r = wqT.rearrange("(kc p) e -> p kc e", p=P)
    wkvT_r = wkvT.rearrange("(kc p) l -> p kc l", p=P)
    wkuT_r = wkuT.rearrange("(lc p) d -> p lc d", p=P)
    wvuT_r = wvuT.rearrange("(lc p) d -> p lc d", p=P)
    woT_r = woT.rearrange("(h p) c -> p h c", p=P)

    with tile.TileContext(nc) as tc, ExitStack() as top:
        persist = top.enter_context(tc.tile_pool(name="persist", bufs=1))
        pall = top.enter_context(tc.tile_pool(name="pall", bufs=1, space="PSUM"))
        xpool = top.enter_context(tc.tile_pool(name="xpool", bufs=2))
        kvpool = top.enter_context(tc.tile_pool(name="kvpool", bufs=1))
        rpool = top.enter_context(tc.tile_pool(name="rpool", bufs=2))
        qtp = top.enter_context(tc.tile_pool(name="qtp", bufs=3))
        epool = top.enter_context(tc.tile_pool(name="epool", bufs=2))
        zpool = top.enter_context(tc.tile_pool(name="zpool", bufs=2))
        ypool = top.enter_context(tc.tile_pool(name="ypool", bufs=2))
        opool = top.enter_context(tc.tile_pool(name="opool", bufs=1))

        def pp_tile(name):
            return pall.tile([P, 2, 512], F32, tag="pp", bufs=3, name=name)

        def ps_tile(name, shape=(P, 512)):
            return pall.tile(list(shape), F32, tag="ps", bufs=2, name=name)

        k_slab = persist.tile([P, NT, P], BF16)      # k_rot^T: [d, ts_chunk, ts]
        v_slab = persist.tile([P, NT, P], BF16)      # v: [ts, ts_chunk, d]
        trim = persist.tile([P, P], BF16)            # 0/1 causal keep mask
        if not USE_GPSIMD_Z:
            ones = persist.tile([P, P], BF16)
            ones_f = persist.tile([P, 1], F32)
        c2_sb = persist.tile([P, T], BF16)
        s2_sb = persist.tile([P, T], BF16)
        wku_sb = persist.tile([P, LC, D], BF16)
        wvu_sb = persist.tile([P, LC, D], BF16)
        wku0_sb = persist.tile([P, LC, D], BF16)
        wvu0_sb = persist.tile([P, LC, D], BF16)
        wq_sb = persist.tile([P, KC, HG * P], BF16)
        wo_sb = persist.tile([P, HG, C], BF16)

        # --- init compute first (cheap; keeps engine queues clear up front)
        # 0/1 keep-mask: 1 where query(col) >= key(part), else 0
        nc.gpsimd.memset(trim[:], 1.0)
        nc.gpsimd.affine_select(
            out=trim[:], in_=trim[:],
            compare_op=mybir.AluOpType.is_ge,
            fill=0.0, base=0,
            pattern=[[1, P]], channel_multiplier=-1,
        )
        if not USE_GPSIMD_Z:
            nc.vector.memset(ones_f[:], 1.0)
            nc.vector.tensor_copy(ones[:], ones_f[:].to_broadcast([P, P]))

        # --- prologue DMAs, spread across the engine queues so the ~0.7us
        # per-dma_start issue overhead parallelizes and the first kv matmuls
        # start as soon as x(g0)+wkv(g0) land. wq slice 0 goes out early on
        # the tensor queue so qproj(h0) isn't starved (needed ~25us in).
        xts = [None] * NQ
        xts[0] = xpool.tile([P, KC, 512], BF16, tag="x", name="x0")
        wkv_sb = persist.tile([P, KC, L], BF16)
        # x0 groups first (they feed the kv-head chain, the first PE work);
        # wq in head-sized column slices AFTER them - slice k arrives just
        # before head k's qproj needs it. wq slice 0 rides the scalar queue
        # early (wkv groups are small) so qproj(h0) is never starved.
        wq_slices = [(0, 128), (128, 256), (256, 512), (512, 768),
                     (768, 1024)]
        for g in range(8):
            nc.sync.dma_start(xts[0][:, bass.ts(g, 2), :],
                              xT_r[:, bass.ts(g, 2), bass.ts(0, 512)])
            nc.scalar.dma_start(wkv_sb[:, bass.ts(g, 2), :],
                                wkvT_r[:, bass.ts(g, 2), :])
            if g == 1:
                a, b = wq_slices[0]
                nc.scalar.dma_start(wq_sb[:, :, a:b], wqT_r[:, :, a:b])
        for a, b in wq_slices[1:]:
            nc.sync.dma_start(wq_sb[:, :, a:b], wqT_r[:, :, a:b])
        nc.gpsimd.dma_start(c2_sb[:, 0:512], c2[:, 0:512])
        nc.gpsimd.dma_start(s2_sb[:, 0:512], s2[:, 0:512])
        nc.gpsimd.dma_start(
            wku0_sb[:], wku0T.rearrange("(lc p) d -> p lc d", p=P))
        nc.gpsimd.dma_start(
            wvu0_sb[:], wvu0T.rearrange("(lc p) d -> p lc d", p=P))
        nc.gpsimd.dma_start(c2_sb[:, 512:T], c2[:, 512:T])
        nc.gpsimd.dma_start(s2_sb[:, 512:T], s2[:, 512:T])
        nc.gpsimd.dma_start(wku_sb[:], wkuT_r)
        nc.gpsimd.dma_start(wvu_sb[:], wvuT_r)
        nc.gpsimd.collective_compute(
            "AllGather", mybir.AluOpType.bypass,
            [[0, 1], [2, 3], [4, 5], [6, 7]], ins=[ccw_s], outs=[ccw_d])

        state = {"pending": None}
        qts = [None] * HG
        kvns = {}

        def rope_copy(ps):
            # single PSUM read via ACT so the bank frees immediately
            qq = rpool.tile([P, 512], BF16, tag="qq")
            nc.scalar.copy(qq[:], ps[:])
            return qq

        def rope_mults(dst, qq, n):
            # dst = qq * c2 + swap64(qq) * s2 on DVE (the two-input TT ops
            # require equal base partitions, so the swap stays a copy)
            c2n = c2_sb[:, bass.ts(n, 512)]
            s2n = s2_sb[:, bass.ts(n, 512)]
            qs = rpool.tile([P, 512], BF16, tag="qs")
            nc.vector.tensor_copy(qs[0:64, :], qq[64:128, :])
            nc.vector.tensor_copy(qs[64:128, :], qq[0:64, :])
            nc.vector.tensor_tensor(qs[:], qs[:], s2n, mybir.AluOpType.mult)
            nc.vector.tensor_tensor(qq[:], qq[:], c2n, mybir.AluOpType.mult)
            nc.vector.tensor_tensor(dst, qq[:], qs[:], mybir.AluOpType.add)

        def rope(dst, ps, n):
            rope_mults(dst, rope_copy(ps), n)

        def flush_zpv(staged=False):
            # staged=True returns (yp-matmul thunks, finalizer) so the PV
            # chain can be spread through the scores tail. The PV inputs are
            # two heads old - zero dep risk.
            if state["pending"] is None:
                return None
            n, h, nts, spans, exp_t, zgm, zsum, y_t = state["pending"]
            state["pending"] = None
            from collections import deque
            yp = ps_tile(f"yp{n}_{h}")

            def mk(j):
                def go():
                    sl = slice(spans[j], 512)
                    nc.tensor.matmul(yp[:, sl], v_slab[:, j, :],
                                     exp_t[:, j, sl],
                                     start=(j == 0), stop=(j == nts - 1))
                return go
            fills = deque(mk(j) for j in range(nts))

            def fin():
                zr = zpool.tile([P, 512], F32, tag="zr", bufs=1)
                if USE_GPSIMD_Z:
                    nc.vector.reciprocal_approx_fast(out=zr[:], in_=zsum[:])
                else:
                    zp = ps_tile(f"zp{n}_{h}")
                    nc.tensor.matmul(zp[:], ones[:], zgm[:], start=True,
                                     stop=True)
                    nc.vector.reciprocal_approx_fast(out=zr[:], in_=zp[:])
                nc.vector.tensor_tensor(y_t[:, h, :], yp[:], zr[:],
                                        mybir.AluOpType.mult)
            if staged:
                return fills, fin
            while fills:
                fills.popleft()()
            fin()
            return None

        def emit_qproj_mms(n, h):
            # q projection chain + the PSUM-freeing ACT copy; the DVE rope
            # multiplies are emitted separately at the end of the iteration so
            # they queue BEHIND the previous head's mask/z-adds on DVE
            qp = ps_tile(f"qp{n}_{h}")
            for kc in range(KC):
                nc.tensor.matmul(qp[:], wq_sb[:, kc, bass.ts(h, P)],
                                 xts[n][:, kc, :],
                                 start=(kc == 0), stop=(kc == KC - 1))
            return rope_copy(qp)

        def emit_rope_q(n, h, qq):
            qt = qtp.tile([P, 512], BF16, tag="qt", name=f"q{n}_{h}")
            rope_mults(qt[:], qq, n)
            qts[h] = qt

        def emit_scores(n, h, y_t):
            nts = 4 * n + 4
            npairs = nts // 2
            spans = [max(P * j - 512 * n, 0) for j in range(nts)]
            exp_t = epool.tile([P, NT, 512], BF16, tag="exp", name=f"e{n}_{h}")
            q_t = qts[h]
            # bf16 partial sums of exp over key blocks on DVE: unmasked pairs
            # as [P,2,512] flat ops, the 4 diagonal blocks into a separate
            # accumulator, folded at the end
            zg2 = zpool.tile([P, 2, 512], BF16, tag="zg2", name=f"zg2_{n}_{h}")
            zgm = zpool.tile([P, 512], BF16, tag="zgm", name=f"zgm{n}_{h}")
            flush_pair = 2 if nts > 4 else 0
            staged = None

            def pump(k):
                nonlocal staged
                if staged is not None:
                    pfills, pfin = staged
                    for _ in range(k):
                        if pfills:
                            pfills.popleft()()
                    if not pfills:
                        pfin()
                        staged = None

            for pi in range(npairs):
                if pi == flush_pair:
                    if flush_pair > 0:
                        staged = flush_zpv(staged=True)
                    else:
                        flush_zpv()
                pg = pp_tile(f"sc{n}_{h}_{pi}")
                for s in range(2):
                    j = 2 * pi + s
                    sl = slice(spans[j], 512)
                    nc.tensor.matmul(pg[:, s, sl], k_slab[:, j, :],
                                     q_t[:, sl], start=True, stop=True)
                    pump(2)
                # one exp per pair, exact-width from the first block's span:
                # columns below it hold garbage and are never read downstream
                g0 = spans[2 * pi]
                nc.scalar.activation(
                    exp_t[:, 2 * pi:2 * pi + 2, g0:512], pg[:, :, g0:512],
                    mybir.ActivationFunctionType.Exp, scale=SCALE)
                if 2 * pi >= 4 * n:
                    # diagonal pair: causal mask as 0/1 multiply on exp output
                    for s in range(2):
                        j = 2 * pi + s
                        g = spans[j]
                        nc.vector.tensor_tensor(
                            exp_t[:, j, g:g + P], exp_t[:, j, g:g + P],
                            trim[:], mybir.AluOpType.mult)
                    for s in range(2):
                        j = 2 * pi + s
                        sl = slice(spans[j], 512)
                        if j == 4 * n:
                            nc.vector.tensor_copy(zgm[:], exp_t[:, j, :])
                        else:
                            nc.vector.tensor_tensor(zgm[:, sl], zgm[:, sl],
                                                    exp_t[:, j, sl],
                                                    mybir.AluOpType.add)
                else:
                    pair = exp_t[:, 2 * pi:2 * pi + 2, :]
                    if pi == 0:
                        nc.vector.tensor_copy(zg2[:], pair)
                    else:
                        nc.vector.tensor_tensor(zg2[:], zg2[:], pair,
                                                mybir.AluOpType.add)
                pump(2)
            if staged is not None:
                pfills, pfin = staged
                while pfills:
                    pfills.popleft()()
                pfin()
                staged = None
            if n > 0:
                nc.vector.tensor_tensor(zgm[:], zgm[:], zg2[:, 0, :],
                                        mybir.AluOpType.add)
                nc.vector.tensor_tensor(zgm[:], zgm[:], zg2[:, 1, :],
                                        mybir.AluOpType.add)
            if flush_pair == 0 and state["pending"] is not None:
                flush_zpv()
            if USE_GPSIMD_Z:
                zsum = zpool.tile([P, 512], F32, tag="zs", bufs=1,
                                  name=f"zs{n}_{h}")
                nc.gpsimd.partition_all_reduce(zsum[:], zgm[:], P,
                                               bass_isa.ReduceOp.add)
            else:
                zsum = None
            state["pending"] = (n, h, nts, spans, exp_t, zgm, zsum, y_t)

        kvps_st = {}

        def emit_kv_head(n, kcs):
            # chunk 0: full latent kv locally; chunks 1-3: only this core's
            # HALF (wkv columns are own-half-first per core) - the pair core
            # computes the other half and an AllGather merges them
            nl = LC if n == 0 else 2
            if n not in kvps_st:
                kvps_st[n] = [pp_tile(f"kv{n}_{i}") for i in range(nl // 2)]
            for kc in kcs:
                for lc in range(nl):
                    nc.tensor.matmul(kvps_st[n][lc // 2][:, lc % 2, :],
                                     wkv_sb[:, kc, bass.ts(lc, P)],
                                     xts[n][:, kc, :],
                                     start=(kc == 0), stop=(kc == KC - 1))

        def emit_kv_stage(n):
            kvps = kvps_st.pop(n)
            kvh = kvpool.tile([P, 2, 512], BF16, tag="kvh")
            nc.scalar.copy(kvh[:], kvps[0][:])
            nc.sync.dma_start(
                ccs[n - 1].rearrange("(s p) t -> p s t", p=P), kvh[:])
            nc.gpsimd.collective_compute(
                "AllGather", mybir.AluOpType.bypass,
                [[0, 1], [2, 3], [4, 5], [6, 7]],
                ins=[ccs[n - 1]], outs=[ccd[n - 1]])
            # preload the gathered latent now so kv_tail(n) has no DMA wait
            kvn = kvpool.tile([P, LC, 512], BF16, tag="kvn", name=f"kvn{n}")
            nc.scalar.dma_start(
                kvn[:], ccd[n - 1].rearrange("b (s p) t -> p (b s) t", p=P))
            kvns[n] = kvn

        def emit_kv_tail(n):
            if n == 0:
                kvn = kvpool.tile([P, LC, 512], BF16, tag="kvn", name="kvn0")
                kvps = kvps_st.pop(0)
                nc.scalar.copy(kvn[:, 0:2, :], kvps[0][:])
                nc.scalar.copy(kvn[:, 2:4, :], kvps[1][:])
                wku, wvu = wku0_sb, wvu0_sb
            else:
                kvn = kvns.pop(n)
                wku, wvu = wku_sb, wvu_sb

            kp = ps_tile(f"kp{n}")
            for lc in range(LC):
                nc.tensor.matmul(kp[:], wku[:, lc, :], kvn[:, lc, :],
                                 start=(lc == 0), stop=(lc == LC - 1))
            kdst = k_slab[:, 4 * n:4 * (n + 1), :].rearrange("p a b -> p (a b)")
            rope(kdst, kp, n)

            vp = ps_tile(f"vp{n}", shape=(P, 4, P))
            for i in range(4):
                for lc in range(LC):
                    nc.tensor.matmul(vp[:, i, :], kvn[:, lc, bass.ts(i, P)],
                                     wvu[:, lc, :],
                                     start=(lc == 0), stop=(lc == LC - 1))
            nc.scalar.copy(v_slab[:, 4 * n:4 * n + 4, :], vp[:])

        def emit_C(n, y_t):
            # h-inner with ci under it: the ci matmuls share the y-chunk
            # stationary; drains split DVE/ACT; output DMA split across two
            # queues per 128-token row block
            for t in range(4):
                oph = [pp_tile(f"op{n}_{t}_{q}") for q in range(2)]
                for h in range(HG):
                    for ci in range(4):
                        nc.tensor.matmul(oph[ci // 2][:, ci % 2, :],
                                         y_t[:, h, bass.ts(t, P)],
                                         wo_sb[:, h, bass.ts(ci, 512)],
                                         start=(h == 0), stop=(h == HG - 1))
                ost = opool.tile([P, 4, 512], BF16, tag="ost")
                nc.vector.tensor_copy(ost[:, 0:2, :], oph[0][:])
                nc.scalar.copy(ost[:, 2:4, :], oph[1][:])
                row = bass.ts(4 * n + t, P)
                nc.sync.dma_start(
                    outp[row, 0:1024],
                    ost[:, 0:2, :].rearrange("p a b -> p (a b)"))
                nc.sync.dma_start(
                    outp[row, 1024:2048],
                    ost[:, 2:4, :].rearrange("p a b -> p (a b)"))

        def run_late_fin():
            f = state.pop("late_fin", None)
            if f is not None:
                f()

        emit_kv_head(0, range(KC))
        for n in range(NQ):
            y_t = ypool.tile([P, HG, 512], BF16, tag="yc", name=f"y{n}")
            for h in range(HG):
                if h == 0:
                    emit_kv_tail(n)
                qq = emit_qproj_mms(n, h)
                # rope right after qproj: its DVE ops can start the moment
                # the ACT copy lands, ahead of this iteration's zg adds and
                # the deferred fin - shortest path to the next scores MMs
                emit_rope_q(n, h, qq)
                if h >= 1:
                    emit_scores(n, h - 1, y_t)
                    run_late_fin()
                if h == 2 and n + 1 < NQ:
                    xts[n + 1] = xpool.tile([P, KC, 512], BF16, tag="x",
                                            name=f"x{n + 1}")
                    nc.sync.dma_start(xts[n + 1][:],
                                      xT_r[:, :, bass.ts(n + 1, 512)])
                if h == 4 and n == 0:
                    for i in range(4):
                        nc.sync.dma_start(wo_sb[:, :, bass.ts(i, C // 4)],
                                          woT_r[:, :, bass.ts(i, C // 4)])
            # tail: first half of kv(n+1) covers the rope(h7) latency before
            # scores(h7); the stage + AllGather launch right after, hiding
            # the collective under the flush, emit_C and the next q chains
            if n + 1 < NQ:
                emit_kv_head(n + 1, range(0, KC // 2))
                emit_scores(n, HG - 1, y_t)
                run_late_fin()
                emit_kv_head(n + 1, range(KC // 2, KC))
                emit_kv_stage(n + 1)
                flush_zpv()
            else:
                emit_scores(n, HG - 1, y_t)
                run_late_fin()
                flush_zpv()
            emit_C(n, y_t)

    nc.finalize()
    return nc


_PERM = np.concatenate([np.arange(0, D, 2), np.arange(1, D, 2)])


def _prep_core_inputs(x, freqs_cos, freqs_sin, wq, wkv_down, wk_up, wv_up, wo):
    cosT = np.ascontiguousarray(freqs_cos.T)                      # [64, T]
    sinT = np.ascontiguousarray(freqs_sin.T)
    c2 = np.concatenate([cosT, cosT], axis=0).astype(NPBF16)      # [128, T]
    s2 = np.concatenate([-sinT, sinT], axis=0).astype(NPBF16)

    wkvT = np.ascontiguousarray(wkv_down.T).astype(NPBF16)        # [C, L]
    wkuT = np.ascontiguousarray(wk_up[_PERM, :].T).astype(NPBF16)  # [L, D]
    wvuT = np.ascontiguousarray(wv_up.T).astype(NPBF16)           # [L, D]

    wq_h = wq.reshape(H, D, C)[:, _PERM, :]                       # perm rows/head

    in_maps = []
    for core in range(N_CORES):
        b, g = core // 2, core % 2
        heads = slice(8 * g, 8 * g + 8)
        wqT_g = np.ascontiguousarray(
            wq_h[heads].reshape(HG * D, C).T).astype(NPBF16)      # [C, 1024]
        wkvT_g = np.ascontiguousarray(
            np.concatenate([wkvT[:, 256 * g:256 * g + 256],
                            wkvT[:, 256 * (1 - g):256 * (1 - g) + 256]],
                           axis=1))
        wku0T_g = np.ascontiguousarray(np.roll(wkuT, -256 * g, axis=0))
        wvu0T_g = np.ascontiguousarray(np.roll(wvuT, -256 * g, axis=0))
        woT_g = np.ascontiguousarray(
            wo[:, 8 * g * D:(8 * g + 8) * D].T).astype(NPBF16)    # [1024, C]
        xT_b = np.ascontiguousarray(x[b].T).astype(NPBF16)        # [C, T]
        in_maps.append({
            "xT": xT_b, "wqT": wqT_g, "wkvT": wkvT_g, "wkuT": wkuT,
            "wvuT": wvuT, "wku0T": wku0T_g, "wvu0T": wvu0T_g,
            "woT": woT_g, "c2": c2, "s2": s2,
        })
    return in_maps


def kernel(x, freqs_cos, freqs_sin, wq, wkv_down, wk_up, wv_up, wo, _trace=False):
    x = np.asarray(x, dtype=np.float32)
    freqs_cos = np.asarray(freqs_cos, dtype=np.float32)
    freqs_sin = np.asarray(freqs_sin, dtype=np.float32)
    wq = np.asarray(wq, dtype=np.float32)
    wkv_down = np.asarray(wkv_down, dtype=np.float32)
    wk_up = np.asarray(wk_up, dtype=np.float32)
    wv_up = np.asarray(wv_up, dtype=np.float32)
    wo = np.asarray(wo, dtype=np.float32)

    if "nc" not in _cached:
        _cached["nc"] = _build_program()
    nc = _cached["nc"]

    in_maps = _prep_core_inputs(x, freqs_cos, freqs_sin, wq, wkv_down,
                                wk_up, wv_up, wo)
    res = run_bass_kernel_spmd(nc, in_maps, core_ids=list(range(N_CORES)),
                               trace=_trace)
    _cached["last_result"] = res

    out = np.empty((B, T, C), dtype=np.float32)
    for b in range(B):
        out[b] = (res.results[2 * b]["outp"].astype(np.float32)
                  + res.results[2 * b + 1]["outp"].astype(np.float32))
    return out


# revision 19
# speedup vs baseline: 1.2143x; 1.0296x over previous
"""MLA (multi-head latent) causal attention on 8 Trainium2 NeuronCores.

Sharding: batch(4) x head-group(2) mesh over 8 cores. Core c handles batch
c//2 and heads [8*(c%2), 8*(c%2)+8). The latent KV projections are small and
recomputed per head-group (an MLA property: the latent KV is shared across
heads). Each core produces a partial output (its head-group's contribution to
y @ wo^T for its batch); the host sums the two partials per batch.

All data is bf16 (rel tolerance 2e-2 leaves plenty of room): this enables the
PE's Fast Weight Load path, halves HBM traffic, and doubles DVE throughput.
PSUM accumulation stays fp32.

Single fused loop over the 4 query chunks of 512 tokens; everything streams
through SBUF (no DRAM scratch):
  per chunk n:
    h-loop (software pipelined): q-proj(h) chain -> RoPE(h) on ACT+DVE,
      scores(h-1) as PAIRS of key blocks into 2-bank PSUM tiles -> one exp
      per pair on ACT -> causal mask as a 0/1 bf16 multiply on the exp
      output (DVE) -> bf16 partial sums of exp on DVE, PV interleaved
      mid-scores, softmax denominator via GPSIMD partition_all_reduce
      (keeps the PE free), 1/z normalize on DVE.
    tail: latent kv proj for chunk n+1 split around the AllGather stage,
      then the output projection with DVE+ACT evacuation in parallel and
      the output DMA split across two queues.

PSUM budget (8 banks): tag "pp" = 3 x [P,2,512] pairs (6 banks, score pairs /
latent halves / out-proj pairs), tag "ps" = 2 x [P,512] singles (q-proj, PV
accumulator, k-up, v-up).
"""

import math
from contextlib import ExitStack

import numpy as np

import concourse.bass as bass
import concourse.mybir as mybir
import concourse.tile as tile
from concourse import bacc
from concourse import bass_isa
from concourse.bass_utils import run_bass_kernel_spmd

# Problem shape (hardcoded per contract).
B, T, C = 4, 2048, 2048
H, D, L = 16, 128, 512
HG = 8           # heads per core
N_CORES = 8
P = 128
KC = C // P      # 16 contraction chunks over C
LC = L // P      # 4 chunks over L
NQ = T // 512    # 4 query chunks of 512
NT = T // P      # 16 key chunks of 128
SCALE = 1.0 / math.sqrt(D)

F32 = mybir.dt.float32
BF16 = mybir.dt.bfloat16
NPBF16 = mybir.dt.np(BF16)

USE_GPSIMD_Z = False  # gpsimd z trips the P0 power throttle (chip drops to 2.0GHz)

_cached = {}


def _build_program():
    nc = bacc.Bacc()

    xT = nc.dram_tensor("xT", [C, T], BF16, kind="ExternalInput").ap()
    wqT = nc.dram_tensor("wqT", [C, HG * D], BF16, kind="ExternalInput").ap()
    wkvT = nc.dram_tensor("wkvT", [C, L], BF16, kind="ExternalInput").ap()
    wkuT = nc.dram_tensor("wkuT", [L, D], BF16, kind="ExternalInput").ap()
    wvuT = nc.dram_tensor("wvuT", [L, D], BF16, kind="ExternalInput").ap()
    woT = nc.dram_tensor("woT", [HG * D, C], BF16, kind="ExternalInput").ap()
    c2 = nc.dram_tensor("c2", [P, T], BF16, kind="ExternalInput").ap()
    s2 = nc.dram_tensor("s2", [P, T], BF16, kind="ExternalInput").ap()
    outp = nc.dram_tensor("outp", [T, C], BF16, kind="ExternalOutput").ap()
    wku0T = nc.dram_tensor("wku0T", [L, D], BF16, kind="ExternalInput").ap()
    wvu0T = nc.dram_tensor("wvu0T", [L, D], BF16, kind="ExternalInput").ap()
    ccw_s = nc.dram_tensor("ccw_s", [P, 16], BF16, kind="Internal").ap()
    ccw_d = nc.dram_tensor("ccw_d", [2, P, 16], BF16, kind="Internal").ap()
    ccs = [nc.dram_tensor(f"ccs{i}", [L // 2, 512], BF16, kind="Internal").ap()
           for i in range(1, NQ)]
    ccd = [nc.dram_tensor(f"ccd{i}", [2, L // 2, 512], BF16,
                          kind="Internal").ap() for i in range(1, NQ)]

    xT_r = xT.rearrange("(kc p) t -> p kc t", p=P)
    wqT_r = wqT.rearrange("(kc p) e -> p kc e", p=P)
    wkvT_r = wkvT.rearrange("(kc p) l -> p kc l", p=P)
    wkuT_r = wkuT.rearrange("(lc p) d -> p lc d", p=P)
    wvuT_r = wvuT.rearrange("(lc p) d -> p lc d", p=P)
    woT_r = woT.rearrange("(h p) c -> p h c", p=P)

    with tile.TileContext(nc) as tc, ExitStack() as top:
        persist = top.enter_context(tc.tile_pool(name="persist", bufs=1))
        pall = top.enter_context(tc.tile_pool(name="pall", bufs=1, space="PSUM"))
        xpool = top.enter_context(tc.tile_pool(name="xpool", bufs=2))
        kvpool = top.enter_context(tc.tile_pool(name="kvpool", bufs=1))
        rpool = top.enter_context(tc.tile_pool(name="rpool", bufs=2))
        qtp = top.enter_context(tc.tile_pool(name="qtp", bufs=3))
        epool = top.enter_context(tc.tile_pool(name="epool", bufs=2))
        zpool = top.enter_context(tc.tile_pool(name="zpool", bufs=2))
        ypool = top.enter_context(tc.tile_pool(name="ypool", bufs=2))
        opool = top.enter_context(tc.tile_pool(name="opool", bufs=1))

        def pp_tile(name):
            return pall.tile([P, 2, 512], F32, tag="pp", bufs=3, name=name)

        def ps_tile(name, shape=(P, 512)):
            return pall.tile(list(shape), F32, tag="ps", bufs=2, name=name)

        k_slab = persist.tile([P, NT, P], BF16)      # k_rot^T: [d, ts_chunk, ts]
        v_slab = persist.tile([P, NT, P], BF16)      # v: [ts, ts_chunk, d]
        trim = persist.tile([P, P], BF16)            # 0/1 causal keep mask
        if not USE_GPSIMD_Z:
            ones = persist.tile([P, P], BF16)
            ones_f = persist.tile([P, 1], F32)
        c2_sb = persist.tile([P, T], BF16)
        s2_sb = persist.tile([P, T], BF16)
        wku_sb = persist.tile([P, LC, D], BF16)
        wvu_sb = persist.tile([P, LC, D], BF16)
        wku0_sb = persist.tile([P, LC, D], BF16)
        wvu0_sb = persist.tile([P, LC, D], BF16)
        wq_sb = persist.tile([P, KC, HG * P], BF16)
        wo_sb = persist.tile([P, HG, C], BF16)

        # --- init compute first (cheap; keeps engine queues clear up front)
        # 0/1 keep-mask: 1 where query(col) >= key(part), else 0
        nc.gpsimd.memset(trim[:], 1.0)
        nc.gpsimd.affine_select(
            out=trim[:], in_=trim[:],
            compare_op=mybir.AluOpType.is_ge,
            fill=0.0, base=0,
            pattern=[[1, P]], channel_multiplier=-1,
        )
        if not USE_GPSIMD_Z:
            nc.vector.memset(ones_f[:], 1.0)
            nc.vector.tensor_copy(ones[:], ones_f[:].to_broadcast([P, P]))

        # --- prologue DMAs, spread across the engine queues so the ~0.7us
        # per-dma_start issue overhead parallelizes and the first kv matmuls
        # start as soon as x(g0)+wkv(g0) land. wq slice 0 goes out early on
        # the tensor queue so qproj(h0) isn't starved (needed ~25us in).
        xts = [None] * NQ
        xts[0] = xpool.tile([P, KC, 512], BF16, tag="x", name="x0")
        wkv_sb = persist.tile([P, KC, L], BF16)
        # x0 groups first (they feed the kv-head chain, the first PE work);
        # wq in 256-col slices AFTER them (256 cols x 2B = 512B runs, the
        # DMA line-rate minimum) - slice k covers heads 2k/2k+1 and arrives
        # roughly when qproj needs it.
        for g in range(8):
            nc.sync.dma_start(xts[0][:, bass.ts(g, 2), :],
                              xT_r[:, bass.ts(g, 2), bass.ts(0, 512)])
            nc.scalar.dma_start(wkv_sb[:, bass.ts(g, 2), :],
                                wkvT_r[:, bass.ts(g, 2), :])
        for i in range(4):
            nc.sync.dma_start(wq_sb[:, :, bass.ts(i, 256)],
                              wqT_r[:, :, bass.ts(i, 256)])
        nc.gpsimd.dma_start(c2_sb[:, 0:512], c2[:, 0:512])
        nc.gpsimd.dma_start(s2_sb[:, 0:512], s2[:, 0:512])
        nc.gpsimd.dma_start(
            wku0_sb[:], wku0T.rearrange("(lc p) d -> p lc d", p=P))
        nc.gpsimd.dma_start(
            wvu0_sb[:], wvu0T.rearrange("(lc p) d -> p lc d", p=P))
        nc.gpsimd.dma_start(c2_sb[:, 512:T], c2[:, 512:T])
        nc.gpsimd.dma_start(s2_sb[:, 512:T], s2[:, 512:T])
        nc.gpsimd.dma_start(wku_sb[:], wkuT_r)
        nc.gpsimd.dma_start(wvu_sb[:], wvuT_r)
        nc.gpsimd.collective_compute(
            "AllGather", mybir.AluOpType.bypass,
            [[0, 1], [2, 3], [4, 5], [6, 7]], ins=[ccw_s], outs=[ccw_d])

        state = {"pending": None}
        qts = [None] * HG
        kvns = {}

        def rope_copy(ps):
            # single PSUM read via ACT so the bank frees immediately
            qq = rpool.tile([P, 512], BF16, tag="qq")
            nc.scalar.copy(qq[:], ps[:])
            return qq

        def rope_mults(dst, qq, n):
            # dst = qq * c2 + swap64(qq) * s2 on DVE (the two-input TT ops
            # require equal base partitions, so the swap stays a copy)
            c2n = c2_sb[:, bass.ts(n, 512)]
            s2n = s2_sb[:, bass.ts(n, 512)]
            qs = rpool.tile([P, 512], BF16, tag="qs")
            nc.vector.tensor_copy(qs[0:64, :], qq[64:128, :])
            nc.vector.tensor_copy(qs[64:128, :], qq[0:64, :])
            nc.vector.tensor_tensor(qs[:], qs[:], s2n, mybir.AluOpType.mult)
            nc.vector.tensor_tensor(qq[:], qq[:], c2n, mybir.AluOpType.mult)
            nc.vector.tensor_tensor(dst, qq[:], qs[:], mybir.AluOpType.add)

        def rope(dst, ps, n):
            rope_mults(dst, rope_copy(ps), n)

        def flush_zpv(staged=False):
            # staged=True returns (yp-matmul thunks, finalizer) so the PV
            # chain can be spread through the scores tail. The PV inputs are
            # two heads old - zero dep risk.
            if state["pending"] is None:
                return None
            n, h, nts, spans, exp_t, zgm, zsum, y_t = state["pending"]
            state["pending"] = None
            from collections import deque
            yp = ps_tile(f"yp{n}_{h}")

            def mk(j):
                def go():
                    sl = slice(spans[j], 512)
                    nc.tensor.matmul(yp[:, sl], v_slab[:, j, :],
                                     exp_t[:, j, sl],
                                     start=(j == 0), stop=(j == nts - 1))
                return go
            fills = deque(mk(j) for j in range(nts))

            def fin():
                zr = zpool.tile([P, 512], F32, tag="zr", bufs=1)
                if USE_GPSIMD_Z:
                    nc.vector.reciprocal_approx_fast(out=zr[:], in_=zsum[:])
                else:
                    # z lives in half of a transient pair tile so the hot
                    # qp/yp "ps" ring never waits on the z->recip chain
                    zp = pp_tile(f"zp{n}_{h}")
                    nc.tensor.matmul(zp[:, 0, :], ones[:], zgm[:],
                                     start=True, stop=True)
                    nc.vector.reciprocal_approx_fast(out=zr[:],
                                                     in_=zp[:, 0, :])
                nc.vector.tensor_tensor(y_t[:, h, :], yp[:], zr[:],
                                        mybir.AluOpType.mult)
            if staged:
                return fills, fin
            while fills:
                fills.popleft()()
            fin()
            return None

        def emit_qproj_mms(n, h):
            # q projection chain + the PSUM-freeing ACT copy; the DVE rope
            # multiplies are emitted separately at the end of the iteration so
            # they queue BEHIND the previous head's mask/z-adds on DVE
            qp = ps_tile(f"qp{n}_{h}")
            for kc in range(KC):
                nc.tensor.matmul(qp[:], wq_sb[:, kc, bass.ts(h, P)],
                                 xts[n][:, kc, :],
                                 start=(kc == 0), stop=(kc == KC - 1))
            return rope_copy(qp)

        def emit_rope_q(n, h, qq):
            qt = qtp.tile([P, 512], BF16, tag="qt", name=f"q{n}_{h}")
            rope_mults(qt[:], qq, n)
            qts[h] = qt

        def emit_scores(n, h, y_t):
            nts = 4 * n + 4
            npairs = nts // 2
            spans = [max(P * j - 512 * n, 0) for j in range(nts)]
            exp_t = epool.tile([P, NT, 512], BF16, tag="exp", name=f"e{n}_{h}")
            q_t = qts[h]
            # bf16 partial sums of exp over key blocks on DVE: unmasked pairs
            # as [P,2,512] flat ops, the 4 diagonal blocks into a separate
            # accumulator, folded at the end
            zg2 = zpool.tile([P, 2, 512], BF16, tag="zg2", name=f"zg2_{n}_{h}")
            zgm = zpool.tile([P, 512], BF16, tag="zgm", name=f"zgm{n}_{h}")
            flush_pair = 2 if nts > 4 else 0
            staged = None

            def pump(k):
                nonlocal staged
                if staged is not None:
                    pfills, pfin = staged
                    for _ in range(k):
                        if pfills:
                            pfills.popleft()()
                    if not pfills:
                        pfin()
                        staged = None

            for pi in range(npairs):
                if pi == flush_pair:
                    if flush_pair > 0:
                        staged = flush_zpv(staged=True)
                    else:
                        flush_zpv()
                pg = pp_tile(f"sc{n}_{h}_{pi}")
                for s in range(2):
                    j = 2 * pi + s
                    sl = slice(spans[j], 512)
                    nc.tensor.matmul(pg[:, s, sl], k_slab[:, j, :],
                                     q_t[:, sl], start=True, stop=True)
                    pump(2)
                # one exp per pair, exact-width from the first block's span:
                # columns below it hold garbage and are never read downstream
                g0 = spans[2 * pi]
                nc.scalar.activation(
                    exp_t[:, 2 * pi:2 * pi + 2, g0:512], pg[:, :, g0:512],
                    mybir.ActivationFunctionType.Exp, scale=SCALE)
                if 2 * pi >= 4 * n:
                    # diagonal pair: causal mask as 0/1 multiply on exp output
                    for s in range(2):
                        j = 2 * pi + s
                        g = spans[j]
                        nc.vector.tensor_tensor(
                            exp_t[:, j, g:g + P], exp_t[:, j, g:g + P],
                            trim[:], mybir.AluOpType.mult)
                    for s in range(2):
                        j = 2 * pi + s
                        sl = slice(spans[j], 512)
                        if j == 4 * n:
                            nc.vector.tensor_copy(zgm[:], exp_t[:, j, :])
                        else:
                            nc.vector.tensor_tensor(zgm[:, sl], zgm[:, sl],
                                                    exp_t[:, j, sl],
                                                    mybir.AluOpType.add)
                else:
                    pair = exp_t[:, 2 * pi:2 * pi + 2, :]
                    if pi == 0:
                        nc.vector.tensor_copy(zg2[:], pair)
                    else:
                        nc.vector.tensor_tensor(zg2[:], zg2[:], pair,
                                                mybir.AluOpType.add)
                pump(2)
            if staged is not None:
                pfills, pfin = staged
                while pfills:
                    pfills.popleft()()
                pfin()
                staged = None
            if n > 0:
                nc.vector.tensor_tensor(zgm[:], zgm[:], zg2[:, 0, :],
                                        mybir.AluOpType.add)
                nc.vector.tensor_tensor(zgm[:], zgm[:], zg2[:, 1, :],
                                        mybir.AluOpType.add)
            if flush_pair == 0 and state["pending"] is not None:
                flush_zpv()
            if USE_GPSIMD_Z:
                zsum = zpool.tile([P, 512], F32, tag="zs", bufs=1,
                                  name=f"zs{n}_{h}")
                nc.gpsimd.partition_all_reduce(zsum[:], zgm[:], P,
                                               bass_isa.ReduceOp.add)
            else:
                zsum = None
            state["pending"] = (n, h, nts, spans, exp_t, zgm, zsum, y_t)

        kvps_st = {}

        def emit_kv_head(n, kcs):
            # chunk 0: full latent kv locally; chunks 1-3: only this core's
            # HALF (wkv columns are own-half-first per core) - the pair core
            # computes the other half and an AllGather merges them
            nl = LC if n == 0 else 2
            if n not in kvps_st:
                kvps_st[n] = [pp_tile(f"kv{n}_{i}") for i in range(nl // 2)]
            for kc in kcs:
                for lc in range(nl):
                    nc.tensor.matmul(kvps_st[n][lc // 2][:, lc % 2, :],
                                     wkv_sb[:, kc, bass.ts(lc, P)],
                                     xts[n][:, kc, :],
                                     start=(kc == 0), stop=(kc == KC - 1))

        def emit_kv_stage(n):
            kvps = kvps_st.pop(n)
            kvh = kvpool.tile([P, 2, 512], BF16, tag="kvh")
            nc.scalar.copy(kvh[:], kvps[0][:])
            nc.sync.dma_start(
                ccs[n - 1].rearrange("(s p) t -> p s t", p=P), kvh[:])
            nc.gpsimd.collective_compute(
                "AllGather", mybir.AluOpType.bypass,
                [[0, 1], [2, 3], [4, 5], [6, 7]],
                ins=[ccs[n - 1]], outs=[ccd[n - 1]])
            # preload the gathered latent now so kv_tail(n) has no DMA wait
            kvn = kvpool.tile([P, LC, 512], BF16, tag="kvn", name=f"kvn{n}")
            nc.scalar.dma_start(
                kvn[:], ccd[n - 1].rearrange("b (s p) t -> p (b s) t", p=P))
            kvns[n] = kvn

        def emit_kv_tail(n):
            if n == 0:
                kvn = kvpool.tile([P, LC, 512], BF16, tag="kvn", name="kvn0")
                kvps = kvps_st.pop(0)
                nc.scalar.copy(kvn[:, 0:2, :], kvps[0][:])
                nc.scalar.copy(kvn[:, 2:4, :], kvps[1][:])
                wku, wvu = wku0_sb, wvu0_sb
            else:
                kvn = kvns.pop(n)
                wku, wvu = wku_sb, wvu_sb

            kp = ps_tile(f"kp{n}")
            for lc in range(LC):
                nc.tensor.matmul(kp[:], wku[:, lc, :], kvn[:, lc, :],
                                 start=(lc == 0), stop=(lc == LC - 1))
            kdst = k_slab[:, 4 * n:4 * (n + 1), :].rearrange("p a b -> p (a b)")
            rope(kdst, kp, n)

            vp = ps_tile(f"vp{n}", shape=(P, 4, P))
            for i in range(4):
                for lc in range(LC):
                    nc.tensor.matmul(vp[:, i, :], kvn[:, lc, bass.ts(i, P)],
                                     wvu[:, lc, :],
                                     start=(lc == 0), stop=(lc == LC - 1))
            nc.scalar.copy(v_slab[:, 4 * n:4 * n + 4, :], vp[:])

        def emit_C(n, y_t):
            # h-inner with ci under it: the ci matmuls share the y-chunk
            # stationary; drains split DVE/ACT; output DMA split across two
            # queues per 128-token row block
            for t in range(4):
                oph = [pp_tile(f"op{n}_{t}_{q}") for q in range(2)]
                for h in range(HG):
                    for ci in range(4):
                        nc.tensor.matmul(oph[ci // 2][:, ci % 2, :],
                                         y_t[:, h, bass.ts(t, P)],
                                         wo_sb[:, h, bass.ts(ci, 512)],
                                         start=(h == 0), stop=(h == HG - 1))
                ost = opool.tile([P, 4, 512], BF16, tag="ost")
                nc.vector.tensor_copy(ost[:, 0:2, :], oph[0][:])
                nc.scalar.copy(ost[:, 2:4, :], oph[1][:])
                row = bass.ts(4 * n + t, P)
                nc.sync.dma_start(
                    outp[row, 0:1024],
                    ost[:, 0:2, :].rearrange("p a b -> p (a b)"))
                nc.sync.dma_start(
                    outp[row, 1024:2048],
                    ost[:, 2:4, :].rearrange("p a b -> p (a b)"))

        def run_late_fin():
            f = state.pop("late_fin", None)
            if f is not None:
                f()

        emit_kv_head(0, range(KC))
        for n in range(NQ):
            y_t = ypool.tile([P, HG, 512], BF16, tag="yc", name=f"y{n}")
            for h in range(HG):
                if h == 0:
                    emit_kv_tail(n)
                qq = emit_qproj_mms(n, h)
                # rope right after qproj: its DVE ops can start the moment
                # the ACT copy lands, ahead of this iteration's zg adds and
                # the deferred fin - shortest path to the next scores MMs
                emit_rope_q(n, h, qq)
                if h >= 1:
                    emit_scores(n, h - 1, y_t)
                    run_late_fin()
                if h == 2 and n + 1 < NQ:
                    xts[n + 1] = xpool.tile([P, KC, 512], BF16, tag="x",
                                            name=f"x{n + 1}")
                    nc.sync.dma_start(xts[n + 1][:],
                                      xT_r[:, :, bass.ts(n + 1, 512)])
                if h == 4 and n == 0:
                    for i in range(4):
                        nc.sync.dma_start(wo_sb[:, :, bass.ts(i, C // 4)],
                                          woT_r[:, :, bass.ts(i, C // 4)])
            # tail: first half of kv(n+1) covers the rope(h7) latency before
            # scores(h7); the stage + AllGather launch right after, hiding
            # the collective under the flush, emit_C and the next q chains
            if n + 1 < NQ:
                emit_kv_head(n + 1, range(0, KC // 2))
                emit_scores(n, HG - 1, y_t)
                run_late_fin()
                emit_kv_head(n + 1, range(KC // 2, KC))
                emit_kv_stage(n + 1)
                flush_zpv()
            else:
                emit_scores(n, HG - 1, y_t)
                run_late_fin()
                flush_zpv()
            emit_C(n, y_t)

    nc.finalize()
    return nc


_PERM = np.concatenate([np.arange(0, D, 2), np.arange(1, D, 2)])


def _prep_core_inputs(x, freqs_cos, freqs_sin, wq, wkv_down, wk_up, wv_up, wo):
    cosT = np.ascontiguousarray(freqs_cos.T)                      # [64, T]
    sinT = np.ascontiguousarray(freqs_sin.T)
    c2 = np.concatenate([cosT, cosT], axis=0).astype(NPBF16)      # [128, T]
    s2 = np.concatenate([-sinT, sinT], axis=0).astype(NPBF16)

    wkvT = np.ascontiguousarray(wkv_down.T).astype(NPBF16)        # [C, L]
    wkuT = np.ascontiguousarray(wk_up[_PERM, :].T).astype(NPBF16)  # [L, D]
    wvuT = np.ascontiguousarray(wv_up.T).astype(NPBF16)           # [L, D]

    wq_h = wq.reshape(H, D, C)[:, _PERM, :]                       # perm rows/head

    in_maps = []
    for core in range(N_CORES):
        b, g = core // 2, core % 2
        heads = slice(8 * g, 8 * g + 8)
        wqT_g = np.ascontiguousarray(
            wq_h[heads].reshape(HG * D, C).T).astype(NPBF16)      # [C, 1024]
        wkvT_g = np.ascontiguousarray(
            np.concatenate([wkvT[:, 256 * g:256 * g + 256],
                            wkvT[:, 256 * (1 - g):256 * (1 - g) + 256]],
                           axis=1))
        wku0T_g = np.ascontiguousarray(np.roll(wkuT, -256 * g, axis=0))
        wvu0T_g = np.ascontiguousarray(np.roll(wvuT, -256 * g, axis=0))
        woT_g = np.ascontiguousarray(
            wo[:, 8 * g * D:(8 * g + 8) * D].T).astype(NPBF16)    # [1024, C]
        xT_b = np.ascontiguousarray(x[b].T).astype(NPBF16)        # [C, T]
        in_maps.append({
            "xT": xT_b, "wqT": wqT_g, "wkvT": wkvT_g, "wkuT": wkuT,
            "wvuT": wvuT, "wku0T": wku0T_g, "wvu0T": wvu0T_g,
            "woT": woT_g, "c2": c2, "s2": s2,
        })
    return in_maps


def kernel(x, freqs_cos, freqs_sin, wq, wkv_down, wk_up, wv_up, wo, _trace=False):
    x = np.asarray(x, dtype=np.float32)
    freqs_cos = np.asarray(freqs_cos, dtype=np.float32)
    freqs_sin = np.asarray(freqs_sin, dtype=np.float32)
    wq = np.asarray(wq, dtype=np.float32)
    wkv_down = np.asarray(wkv_down, dtype=np.float32)
    wk_up = np.asarray(wk_up, dtype=np.float32)
    wv_up = np.asarray(wv_up, dtype=np.float32)
    wo = np.asarray(wo, dtype=np.float32)

    if "nc" not in _cached:
        _cached["nc"] = _build_program()
    nc = _cached["nc"]

    in_maps = _prep_core_inputs(x, freqs_cos, freqs_sin, wq, wkv_down,
                                wk_up, wv_up, wo)
    res = run_bass_kernel_spmd(nc, in_maps, core_ids=list(range(N_CORES)),
                               trace=_trace)
    _cached["last_result"] = res

    out = np.empty((B, T, C), dtype=np.float32)
    for b in range(B):
        out[b] = (res.results[2 * b]["outp"].astype(np.float32)
                  + res.results[2 * b + 1]["outp"].astype(np.float32))
    return out


# revision 22
# speedup vs baseline: 1.2170x; 1.0023x over previous
"""MLA (multi-head latent) causal attention on 8 Trainium2 NeuronCores.

Sharding: batch(4) x head-group(2) mesh over 8 cores. Core c handles batch
c//2 and heads [8*(c%2), 8*(c%2)+8). The latent KV projections are small and
recomputed per head-group (an MLA property: the latent KV is shared across
heads). Each core produces a partial output (its head-group's contribution to
y @ wo^T for its batch); the host sums the two partials per batch.

All data is bf16 (rel tolerance 2e-2 leaves plenty of room): this enables the
PE's Fast Weight Load path, halves HBM traffic, and doubles DVE throughput.
PSUM accumulation stays fp32.

Single fused loop over the 4 query chunks of 512 tokens; everything streams
through SBUF (no DRAM scratch):
  per chunk n:
    h-loop (software pipelined): q-proj(h) chain -> RoPE(h) on ACT+DVE,
      scores(h-1) as PAIRS of key blocks into 2-bank PSUM tiles -> one exp
      per pair on ACT -> causal mask as a 0/1 bf16 multiply on the exp
      output (DVE) -> bf16 partial sums of exp on DVE, PV interleaved
      mid-scores, softmax denominator via GPSIMD partition_all_reduce
      (keeps the PE free), 1/z normalize on DVE.
    tail: latent kv proj for chunk n+1 split around the AllGather stage,
      then the output projection with DVE+ACT evacuation in parallel and
      the output DMA split across two queues.

PSUM budget (8 banks): tag "pp" = 3 x [P,2,512] pairs (6 banks, score pairs /
latent halves / out-proj pairs), tag "ps" = 2 x [P,512] singles (q-proj, PV
accumulator, k-up, v-up).
"""

import math
from contextlib import ExitStack

import numpy as np

import concourse.bass as bass
import concourse.mybir as mybir
import concourse.tile as tile
from concourse import bacc
from concourse import bass_isa
from concourse.bass_utils import run_bass_kernel_spmd

# Problem shape (hardcoded per contract).
B, T, C = 4, 2048, 2048
H, D, L = 16, 128, 512
HG = 8           # heads per core
N_CORES = 8
P = 128
KC = C // P      # 16 contraction chunks over C
LC = L // P      # 4 chunks over L
NQ = T // 512    # 4 query chunks of 512
NT = T // P      # 16 key chunks of 128
SCALE = 1.0 / math.sqrt(D)

F32 = mybir.dt.float32
BF16 = mybir.dt.bfloat16
NPBF16 = mybir.dt.np(BF16)

USE_GPSIMD_Z = False  # gpsimd z trips the P0 power throttle (chip drops to 2.0GHz)

_cached = {}


def _build_program():
    nc = bacc.Bacc()

    xT = nc.dram_tensor("xT", [C, T], BF16, kind="ExternalInput").ap()
    wqT = nc.dram_tensor("wqT", [C, HG * D], BF16, kind="ExternalInput").ap()
    wkvT = nc.dram_tensor("wkvT", [C, L], BF16, kind="ExternalInput").ap()
    wkuT = nc.dram_tensor("wkuT", [L, D], BF16, kind="ExternalInput").ap()
    wvuT = nc.dram_tensor("wvuT", [L, D], BF16, kind="ExternalInput").ap()
    woT = nc.dram_tensor("woT", [HG * D, C], BF16, kind="ExternalInput").ap()
    c2 = nc.dram_tensor("c2", [P, T], BF16, kind="ExternalInput").ap()
    s2 = nc.dram_tensor("s2", [P, T], BF16, kind="ExternalInput").ap()
    outp = nc.dram_tensor("outp", [T, C], BF16, kind="ExternalOutput").ap()
    wku0T = nc.dram_tensor("wku0T", [L, D], BF16, kind="ExternalInput").ap()
    wvu0T = nc.dram_tensor("wvu0T", [L, D], BF16, kind="ExternalInput").ap()
    ccw_s = nc.dram_tensor("ccw_s", [P, 16], BF16, kind="Internal").ap()
    ccw_d = nc.dram_tensor("ccw_d", [2, P, 16], BF16, kind="Internal").ap()
    ccs = [nc.dram_tensor(f"ccs{i}", [L // 2, 512], BF16, kind="Internal").ap()
           for i in range(1, NQ)]
    ccd = [nc.dram_tensor(f"ccd{i}", [2, L // 2, 512], BF16,
                          kind="Internal").ap() for i in range(1, NQ)]

    xT_r = xT.rearrange("(kc p) t -> p kc t", p=P)
    wqT_r = wqT.rearrange("(kc p) e -> p kc e", p=P)
    wkvT_r = wkvT.rearrange("(kc p) l -> p kc l", p=P)
    wkuT_r = wkuT.rearrange("(lc p) d -> p lc d", p=P)
    wvuT_r = wvuT.rearrange("(lc p) d -> p lc d", p=P)
    woT_r = woT.rearrange("(h p) c -> p h c", p=P)

    with tile.TileContext(nc) as tc, ExitStack() as top:
        persist = top.enter_context(tc.tile_pool(name="persist", bufs=1))
        pall = top.enter_context(tc.tile_pool(name="pall", bufs=1, space="PSUM"))
        xpool = top.enter_context(tc.tile_pool(name="xpool", bufs=2))
        kvpool = top.enter_context(tc.tile_pool(name="kvpool", bufs=1))
        rpool = top.enter_context(tc.tile_pool(name="rpool", bufs=2))
        qtp = top.enter_context(tc.tile_pool(name="qtp", bufs=3))
        epool = top.enter_context(tc.tile_pool(name="epool", bufs=2))
        zpool = top.enter_context(tc.tile_pool(name="zpool", bufs=2))
        ypool = top.enter_context(tc.tile_pool(name="ypool", bufs=2))
        opool = top.enter_context(tc.tile_pool(name="opool", bufs=1))

        def pp_tile(name):
            return pall.tile([P, 2, 512], F32, tag="pp", bufs=3, name=name)

        def ps_tile(name, shape=(P, 512)):
            return pall.tile(list(shape), F32, tag="ps", bufs=2, name=name)

        k_slab = persist.tile([P, NT, P], BF16)      # k_rot^T: [d, ts_chunk, ts]
        v_slab = persist.tile([P, NT, P], BF16)      # v: [ts, ts_chunk, d]
        trim = persist.tile([P, P], BF16)            # 0/1 causal keep mask
        if not USE_GPSIMD_Z:
            ones = persist.tile([P, P], BF16)
            ones_f = persist.tile([P, 1], F32)
        c2_sb = persist.tile([P, T], BF16)
        s2_sb = persist.tile([P, T], BF16)
        wku_sb = persist.tile([P, LC, D], BF16)
        wvu_sb = persist.tile([P, LC, D], BF16)
        wku0_sb = persist.tile([P, LC, D], BF16)
        wvu0_sb = persist.tile([P, LC, D], BF16)
        wq_sb = persist.tile([P, KC, HG * P], BF16)
        wo_sb = persist.tile([P, HG, C], BF16)

        # --- init compute first (cheap; keeps engine queues clear up front)
        # 0/1 keep-mask: 1 where query(col) >= key(part), else 0
        nc.gpsimd.memset(trim[:], 1.0)
        nc.gpsimd.affine_select(
            out=trim[:], in_=trim[:],
            compare_op=mybir.AluOpType.is_ge,
            fill=0.0, base=0,
            pattern=[[1, P]], channel_multiplier=-1,
        )
        if not USE_GPSIMD_Z:
            nc.vector.memset(ones_f[:], 1.0)
            nc.vector.tensor_copy(ones[:], ones_f[:].to_broadcast([P, P]))

        # --- prologue DMAs, spread across the engine queues so the ~0.7us
        # per-dma_start issue overhead parallelizes and the first kv matmuls
        # start as soon as x(g0)+wkv(g0) land. wq slice 0 goes out early on
        # the tensor queue so qproj(h0) isn't starved (needed ~25us in).
        xts = [None] * NQ
        xts[0] = xpool.tile([P, KC, 512], BF16, tag="x", name="x0")
        wkv_sb = persist.tile([P, KC, L], BF16)
        # x0 groups first (they feed the kv-head chain, the first PE work);
        # wq in 256-col slices AFTER them (256 cols x 2B = 512B runs, the
        # DMA line-rate minimum) - slice k covers heads 2k/2k+1 and arrives
        # roughly when qproj needs it.
        for g in range(8):
            nc.sync.dma_start(xts[0][:, bass.ts(g, 2), :],
                              xT_r[:, bass.ts(g, 2), bass.ts(0, 512)])
            nc.scalar.dma_start(wkv_sb[:, bass.ts(g, 2), :],
                                wkvT_r[:, bass.ts(g, 2), :])
        for i in range(4):
            nc.sync.dma_start(wq_sb[:, :, bass.ts(i, 256)],
                              wqT_r[:, :, bass.ts(i, 256)])

        # HAM prewarm: ~12 junk matmuls during the x0 DMA wait flip the PE
        # clock gate to 8/8 (~3.4us of sustained activity) so the real kv
        # chain starts at 2.4GHz instead of paying the cold ramp. Inputs are
        # uninitialized k_slab regions (never read downstream; rope-k writes
        # them later, so only a cheap anti-dependency).
        pgw = pp_tile("warm")
        wsrc = k_slab[:, 0:4, :].rearrange("p a b -> p (a b)")
        for _ in range(12):
            nc.tensor.matmul(pgw[:, 0, :], k_slab[:, 15, :], wsrc,
                             start=True, stop=True)
        nc.gpsimd.dma_start(c2_sb[:, 0:512], c2[:, 0:512])
        nc.gpsimd.dma_start(s2_sb[:, 0:512], s2[:, 0:512])
        nc.gpsimd.dma_start(
            wku0_sb[:], wku0T.rearrange("(lc p) d -> p lc d", p=P))
        nc.gpsimd.dma_start(
            wvu0_sb[:], wvu0T.rearrange("(lc p) d -> p lc d", p=P))
        nc.gpsimd.dma_start(c2_sb[:, 512:T], c2[:, 512:T])
        nc.gpsimd.dma_start(s2_sb[:, 512:T], s2[:, 512:T])
        nc.gpsimd.dma_start(wku_sb[:], wkuT_r)
        nc.gpsimd.dma_start(wvu_sb[:], wvuT_r)
        nc.gpsimd.collective_compute(
            "AllGather", mybir.AluOpType.bypass,
            [[0, 1], [2, 3], [4, 5], [6, 7]], ins=[ccw_s], outs=[ccw_d])

        state = {"pending": None}
        qts = [None] * HG
        kvns = {}

        def rope_copy(ps):
            # single PSUM read via ACT so the bank frees immediately
            qq = rpool.tile([P, 512], BF16, tag="qq")
            nc.scalar.copy(qq[:], ps[:])
            return qq

        def rope_mults(dst, qq, n):
            # dst = qq * c2 + swap64(qq) * s2 on DVE (the two-input TT ops
            # require equal base partitions, so the swap stays a copy)
            c2n = c2_sb[:, bass.ts(n, 512)]
            s2n = s2_sb[:, bass.ts(n, 512)]
            qs = rpool.tile([P, 512], BF16, tag="qs")
            nc.vector.tensor_copy(qs[0:64, :], qq[64:128, :])
            nc.vector.tensor_copy(qs[64:128, :], qq[0:64, :])
            nc.vector.tensor_tensor(qs[:], qs[:], s2n, mybir.AluOpType.mult)
            nc.vector.tensor_tensor(qq[:], qq[:], c2n, mybir.AluOpType.mult)
            nc.vector.tensor_tensor(dst, qq[:], qs[:], mybir.AluOpType.add)

        def rope(dst, ps, n):
            rope_mults(dst, rope_copy(ps), n)

        def flush_zpv(staged=False):
            # staged=True returns (yp-matmul thunks, finalizer) so the PV
            # chain can be spread through the scores tail. The PV inputs are
            # two heads old - zero dep risk.
            if state["pending"] is None:
                return None
            n, h, nts, spans, exp_t, zgm, zsum, y_t = state["pending"]
            state["pending"] = None
            from collections import deque
            yp = ps_tile(f"yp{n}_{h}")

            def mk(j):
                def go():
                    sl = slice(spans[j], 512)
                    nc.tensor.matmul(yp[:, sl], v_slab[:, j, :],
                                     exp_t[:, j, sl],
                                     start=(j == 0), stop=(j == nts - 1))
                return go
            fills = deque(mk(j) for j in range(nts))

            def fin():
                zr = zpool.tile([P, 512], F32, tag="zr", bufs=1)
                if USE_GPSIMD_Z:
                    nc.vector.reciprocal_approx_fast(out=zr[:], in_=zsum[:])
                else:
                    # z lives in half of a transient pair tile so the hot
                    # qp/yp "ps" ring never waits on the z->recip chain
                    zp = pp_tile(f"zp{n}_{h}")
                    nc.tensor.matmul(zp[:, 0, :], ones[:], zgm[:],
                                     start=True, stop=True)
                    nc.vector.reciprocal_approx_fast(out=zr[:],
                                                     in_=zp[:, 0, :])
                nc.vector.tensor_tensor(y_t[:, h, :], yp[:], zr[:],
                                        mybir.AluOpType.mult)
            if staged:
                return fills, fin
            while fills:
                fills.popleft()()
            fin()
            return None

        def emit_qproj_mms(n, h):
            # q projection chain + the PSUM-freeing ACT copy; the DVE rope
            # multiplies are emitted separately at the end of the iteration so
            # they queue BEHIND the previous head's mask/z-adds on DVE
            qp = ps_tile(f"qp{n}_{h}")
            for kc in range(KC):
                nc.tensor.matmul(qp[:], wq_sb[:, kc, bass.ts(h, P)],
                                 xts[n][:, kc, :],
                                 start=(kc == 0), stop=(kc == KC - 1))
            return rope_copy(qp)

        def emit_rope_q(n, h, qq):
            qt = qtp.tile([P, 512], BF16, tag="qt", name=f"q{n}_{h}")
            rope_mults(qt[:], qq, n)
            qts[h] = qt

        def emit_scores(n, h, y_t):
            nts = 4 * n + 4
            npairs = nts // 2
            spans = [max(P * j - 512 * n, 0) for j in range(nts)]
            exp_t = epool.tile([P, NT, 512], BF16, tag="exp", name=f"e{n}_{h}")
            q_t = qts[h]
            # bf16 partial sums of exp over key blocks on DVE: unmasked pairs
            # as [P,2,512] flat ops, the 4 diagonal blocks into a separate
            # accumulator, folded at the end
            zg2 = zpool.tile([P, 2, 512], BF16, tag="zg2", name=f"zg2_{n}_{h}")
            zgm = zpool.tile([P, 512], BF16, tag="zgm", name=f"zgm{n}_{h}")
            flush_pair = 2 if nts > 4 else 0
            staged = None

            def pump(k):
                nonlocal staged
                if staged is not None:
                    pfills, pfin = staged
                    for _ in range(k):
                        if pfills:
                            pfills.popleft()()
                    if not pfills:
                        pfin()
                        staged = None

            for pi in range(npairs):
                if pi == flush_pair:
                    if flush_pair > 0:
                        staged = flush_zpv(staged=True)
                    else:
                        flush_zpv()
                pg = pp_tile(f"sc{n}_{h}_{pi}")
                for s in range(2):
                    j = 2 * pi + s
                    sl = slice(spans[j], 512)
                    nc.tensor.matmul(pg[:, s, sl], k_slab[:, j, :],
                                     q_t[:, sl], start=True, stop=True)
                    pump(2)
                # one exp per pair, exact-width from the first block's span:
                # columns below it hold garbage and are never read downstream
                g0 = spans[2 * pi]
                nc.scalar.activation(
                    exp_t[:, 2 * pi:2 * pi + 2, g0:512], pg[:, :, g0:512],
                    mybir.ActivationFunctionType.Exp, scale=SCALE)
                if 2 * pi >= 4 * n:
                    # diagonal pair: causal mask as 0/1 multiply on exp output
                    for s in range(2):
                        j = 2 * pi + s
                        g = spans[j]
                        nc.vector.tensor_tensor(
                            exp_t[:, j, g:g + P], exp_t[:, j, g:g + P],
                            trim[:], mybir.AluOpType.mult)
                    for s in range(2):
                        j = 2 * pi + s
                        sl = slice(spans[j], 512)
                        if j == 4 * n:
                            nc.vector.tensor_copy(zgm[:], exp_t[:, j, :])
                        else:
                            nc.vector.tensor_tensor(zgm[:, sl], zgm[:, sl],
                                                    exp_t[:, j, sl],
                                                    mybir.AluOpType.add)
                else:
                    pair = exp_t[:, 2 * pi:2 * pi + 2, :]
                    if pi == 0:
                        nc.vector.tensor_copy(zg2[:], pair)
                    else:
                        nc.vector.tensor_tensor(zg2[:], zg2[:], pair,
                                                mybir.AluOpType.add)
                pump(2)
            if staged is not None:
                pfills, pfin = staged
                while pfills:
                    pfills.popleft()()
                pfin()
                staged = None
            if n > 0:
                nc.vector.tensor_tensor(zgm[:], zgm[:], zg2[:, 0, :],
                                        mybir.AluOpType.add)
                nc.vector.tensor_tensor(zgm[:], zgm[:], zg2[:, 1, :],
                                        mybir.AluOpType.add)
            if flush_pair == 0 and state["pending"] is not None:
                flush_zpv()
            if USE_GPSIMD_Z:
                zsum = zpool.tile([P, 512], F32, tag="zs", bufs=1,
                                  name=f"zs{n}_{h}")
                nc.gpsimd.partition_all_reduce(zsum[:], zgm[:], P,
                                               bass_isa.ReduceOp.add)
            else:
                zsum = None
            state["pending"] = (n, h, nts, spans, exp_t, zgm, zsum, y_t)

        kvps_st = {}

        def emit_kv_head(n, kcs):
            # chunk 0: full latent kv locally; chunks 1-3: only this core's
            # HALF (wkv columns are own-half-first per core) - the pair core
            # computes the other half and an AllGather merges them
            nl = LC if n == 0 else 2
            if n not in kvps_st:
                kvps_st[n] = [pp_tile(f"kv{n}_{i}") for i in range(nl // 2)]
            for kc in kcs:
                for lc in range(nl):
                    nc.tensor.matmul(kvps_st[n][lc // 2][:, lc % 2, :],
                                     wkv_sb[:, kc, bass.ts(lc, P)],
                                     xts[n][:, kc, :],
                                     start=(kc == 0), stop=(kc == KC - 1))

        def emit_kv_stage(n):
            kvps = kvps_st.pop(n)
            kvh = kvpool.tile([P, 2, 512], BF16, tag="kvh")
            nc.scalar.copy(kvh[:], kvps[0][:])
            nc.sync.dma_start(
                ccs[n - 1].rearrange("(s p) t -> p s t", p=P), kvh[:])
            nc.gpsimd.collective_compute(
                "AllGather", mybir.AluOpType.bypass,
                [[0, 1], [2, 3], [4, 5], [6, 7]],
                ins=[ccs[n - 1]], outs=[ccd[n - 1]])
            # preload the gathered latent now so kv_tail(n) has no DMA wait
            kvn = kvpool.tile([P, LC, 512], BF16, tag="kvn", name=f"kvn{n}")
            nc.scalar.dma_start(
                kvn[:], ccd[n - 1].rearrange("b (s p) t -> p (b s) t", p=P))
            kvns[n] = kvn

        def emit_kv_tail(n):
            if n == 0:
                kvn = kvpool.tile([P, LC, 512], BF16, tag="kvn", name="kvn0")
                kvps = kvps_st.pop(0)
                nc.scalar.copy(kvn[:, 0:2, :], kvps[0][:])
                nc.scalar.copy(kvn[:, 2:4, :], kvps[1][:])
                wku, wvu = wku0_sb, wvu0_sb
            else:
                kvn = kvns.pop(n)
                wku, wvu = wku_sb, wvu_sb

            kp = ps_tile(f"kp{n}")
            for lc in range(LC):
                nc.tensor.matmul(kp[:], wku[:, lc, :], kvn[:, lc, :],
                                 start=(lc == 0), stop=(lc == LC - 1))
            kdst = k_slab[:, 4 * n:4 * (n + 1), :].rearrange("p a b -> p (a b)")
            rope(kdst, kp, n)

            vp = ps_tile(f"vp{n}", shape=(P, 4, P))
            for i in range(4):
                for lc in range(LC):
                    nc.tensor.matmul(vp[:, i, :], kvn[:, lc, bass.ts(i, P)],
                                     wvu[:, lc, :],
                                     start=(lc == 0), stop=(lc == LC - 1))
            nc.scalar.copy(v_slab[:, 4 * n:4 * n + 4, :], vp[:])

        def emit_C(n, y_t):
            # h-inner with ci under it: the ci matmuls share the y-chunk
            # stationary; drains split DVE/ACT; output DMA split across two
            # queues per 128-token row block
            for t in range(4):
                oph = [pp_tile(f"op{n}_{t}_{q}") for q in range(2)]
                for h in range(HG):
                    for ci in range(4):
                        nc.tensor.matmul(oph[ci // 2][:, ci % 2, :],
                                         y_t[:, h, bass.ts(t, P)],
                                         wo_sb[:, h, bass.ts(ci, 512)],
                                         start=(h == 0), stop=(h == HG - 1))
                ost = opool.tile([P, 4, 512], BF16, tag="ost")
                row = bass.ts(4 * n + t, P)
                if n == NQ - 1 and t == 3:
                    # very last block: per-ci copies and DMAs, alternating
                    # engines, so the exposed tail is one small copy + 128KB
                    for ci in range(4):
                        if ci % 2 == 0:
                            nc.vector.tensor_copy(ost[:, ci, :],
                                                  oph[ci // 2][:, ci % 2, :])
                        else:
                            nc.scalar.copy(ost[:, ci, :],
                                           oph[ci // 2][:, ci % 2, :])
                        deng = nc.sync if ci % 2 == 0 else nc.scalar
                        deng.dma_start(outp[row, 512 * ci:512 * (ci + 1)],
                                       ost[:, ci, :])
                else:
                    nc.vector.tensor_copy(ost[:, 0:2, :], oph[0][:])
                    nc.scalar.copy(ost[:, 2:4, :], oph[1][:])
                    nc.sync.dma_start(
                        outp[row, 0:1024],
                        ost[:, 0:2, :].rearrange("p a b -> p (a b)"))
                    nc.sync.dma_start(
                        outp[row, 1024:2048],
                        ost[:, 2:4, :].rearrange("p a b -> p (a b)"))

        def run_late_fin():
            f = state.pop("late_fin", None)
            if f is not None:
                f()

        emit_kv_head(0, range(KC))
        for n in range(NQ):
            y_t = ypool.tile([P, HG, 512], BF16, tag="yc", name=f"y{n}")
            for h in range(HG):
                if h == 0:
                    emit_kv_tail(n)
                qq = emit_qproj_mms(n, h)
                # rope right after qproj: its DVE ops can start the moment
                # the ACT copy lands, ahead of this iteration's zg adds and
                # the deferred fin - shortest path to the next scores MMs
                emit_rope_q(n, h, qq)
                if h >= 1:
                    emit_scores(n, h - 1, y_t)
                    run_late_fin()
                if h == 2 and n + 1 < NQ:
                    xts[n + 1] = xpool.tile([P, KC, 512], BF16, tag="x",
                                            name=f"x{n + 1}")
                    nc.sync.dma_start(xts[n + 1][:],
                                      xT_r[:, :, bass.ts(n + 1, 512)])
                if h == 4 and n == 0:
                    for i in range(4):
                        nc.sync.dma_start(wo_sb[:, :, bass.ts(i, C // 4)],
                                          woT_r[:, :, bass.ts(i, C // 4)])
            # tail: first half of kv(n+1) covers the rope(h7) latency before
            # scores(h7); the stage + AllGather launch right after, hiding
            # the collective under the flush, emit_C and the next q chains
            if n + 1 < NQ:
                emit_kv_head(n + 1, range(0, KC // 2))
                emit_scores(n, HG - 1, y_t)
                run_late_fin()
                emit_kv_head(n + 1, range(KC // 2, KC))
                emit_kv_stage(n + 1)
                flush_zpv()
            else:
                emit_scores(n, HG - 1, y_t)
                run_late_fin()
                flush_zpv()
            emit_C(n, y_t)

    nc.finalize()
    return nc


_PERM = np.concatenate([np.arange(0, D, 2), np.arange(1, D, 2)])


def _prep_core_inputs(x, freqs_cos, freqs_sin, wq, wkv_down, wk_up, wv_up, wo):
    cosT = np.ascontiguousarray(freqs_cos.T)                      # [64, T]
    sinT = np.ascontiguousarray(freqs_sin.T)
    c2 = np.concatenate([cosT, cosT], axis=0).astype(NPBF16)      # [128, T]
    s2 = np.concatenate([-sinT, sinT], axis=0).astype(NPBF16)

    wkvT = np.ascontiguousarray(wkv_down.T).astype(NPBF16)        # [C, L]
    wkuT = np.ascontiguousarray(wk_up[_PERM, :].T).astype(NPBF16)  # [L, D]
    wvuT = np.ascontiguousarray(wv_up.T).astype(NPBF16)           # [L, D]

    wq_h = wq.reshape(H, D, C)[:, _PERM, :]                       # perm rows/head

    in_maps = []
    for core in range(N_CORES):
        b, g = core // 2, core % 2
        heads = slice(8 * g, 8 * g + 8)
        wqT_g = np.ascontiguousarray(
            wq_h[heads].reshape(HG * D, C).T).astype(NPBF16)      # [C, 1024]
        wkvT_g = np.ascontiguousarray(
            np.concatenate([wkvT[:, 256 * g:256 * g + 256],
                            wkvT[:, 256 * (1 - g):256 * (1 - g) + 256]],
                           axis=1))
        wku0T_g = np.ascontiguousarray(np.roll(wkuT, -256 * g, axis=0))
        wvu0T_g = np.ascontiguousarray(np.roll(wvuT, -256 * g, axis=0))
        woT_g = np.ascontiguousarray(
            wo[:, 8 * g * D:(8 * g + 8) * D].T).astype(NPBF16)    # [1024, C]
        xT_b = np.ascontiguousarray(x[b].T).astype(NPBF16)        # [C, T]
        in_maps.append({
            "xT": xT_b, "wqT": wqT_g, "wkvT": wkvT_g, "wkuT": wkuT,
            "wvuT": wvuT, "wku0T": wku0T_g, "wvu0T": wvu0T_g,
            "woT": woT_g, "c2": c2, "s2": s2,
        })
    return in_maps


def kernel(x, freqs_cos, freqs_sin, wq, wkv_down, wk_up, wv_up, wo, _trace=False):
    x = np.asarray(x, dtype=np.float32)
    freqs_cos = np.asarray(freqs_cos, dtype=np.float32)
    freqs_sin = np.asarray(freqs_sin, dtype=np.float32)
    wq = np.asarray(wq, dtype=np.float32)
    wkv_down = np.asarray(wkv_down, dtype=np.float32)
    wk_up = np.asarray(wk_up, dtype=np.float32)
    wv_up = np.asarray(wv_up, dtype=np.float32)
    wo = np.asarray(wo, dtype=np.float32)

    if "nc" not in _cached:
        _cached["nc"] = _build_program()
    nc = _cached["nc"]

    in_maps = _prep_core_inputs(x, freqs_cos, freqs_sin, wq, wkv_down,
                                wk_up, wv_up, wo)
    res = run_bass_kernel_spmd(nc, in_maps, core_ids=list(range(N_CORES)),
                               trace=_trace)
    _cached["last_result"] = res

    out = np.empty((B, T, C), dtype=np.float32)
    for b in range(B):
        out[b] = (res.results[2 * b]["outp"].astype(np.float32)
                  + res.results[2 * b + 1]["outp"].astype(np.float32))
    return out
